# revision 15
# baseline (speedup 1.0000x reference)
"""Trainium2 Bass kernel for nn_ArmaNet02 (ARMA GNN, N=100K, E=1.6M, K=3, T=4, H=16).

Strategy (8 NeuronCores, SPMD), v2:
- dst-sharded ELL, wide gathers use bf16 QUADS (4 nodes x 64 cols = 512B,
  int16 quad indices) from the replicated feature table [NT, 64].
- Narrow state (conv1 t=0 input and the conv2 Horner chain) lives in a
  dedicated compact table [NT, 8] fp32; gather elements are OCTs (8 nodes x
  32B = 256B, single int16 index space).  Horner flushes/AllGathers move
  3.2MB instead of 12.8MB, and the initial x~ table is host-provided so the
  first flush+AllGather disappears.
- gcn_norm folded: dis[src] into table writes, dis[dst] into the edge masks
  (dis computed on host; no on-device degree phase).
"""

import os
import sys
import types

import numpy as np
from ml_dtypes import bfloat16

# ----------------------------------------------------------------------------
# problem constants (hardcoded; kernel.py must be self-contained)
N = 100000
E = 1600000
K = 3
T = 4
H = 16
BN_EPS = 1e-5
NCORE = 8
SHARD = 12500
TPC = 98                 # tiles per core (12544 rows)
ROWS = TPC * 128         # 12544
NT = NCORE * ROWS        # 100352 table rows
NQUAD = NT // 4          # 25088 quad elements (wide)
NOCT = NT // 8           # 12544 oct elements (narrow table)
CPC = 8                  # columns per gather call (8*128 = 1024 slots)
CALL = CPC * 128
IW = CALL // 16          # idx words per call per partition
SEGCAP = 88              # max data columns per segment (pre-padding)

_EXEC_NS = [None]


def _install_hookshim():
    if "antenv.axon_hooks" in sys.modules:
        return
    try:
        import antenv
    except ImportError:
        return
    mod = types.ModuleType("antenv.axon_hooks")
    mod._hook = None
    mod.set_axon_ntff_profile_hook = lambda h: setattr(mod, "_hook", h)
    mod.get_axon_ntff_profile_hook = lambda: mod._hook
    sys.modules["antenv.axon_hooks"] = mod
    antenv.axon_hooks = mod
    try:
        from trn_agent_boot.trn_boot import _ntff_profile_via_ctypes
        hook = _ntff_profile_via_ctypes("/opt/axon/libaxon_pjrt.so")
        if hook is not None:
            mod.set_axon_ntff_profile_hook(hook)
    except Exception:
        pass


# ----------------------------------------------------------------------------
def _build_layout(cnt_all):
    """cnt_all: [8, ROWS] per-dst edge counts in rank order.  Shared layout."""
    D = np.zeros(TPC, np.int64)
    for c in range(NCORE):
        for t in range(TPC):
            D[t] = max(D[t], cnt_all[c][t * 128:(t + 1) * 128].max(initial=0))
    ds = [(t, int(D[t])) for t in range(TPC) if D[t] > 0]
    n = len(ds)
    CAP = SEGCAP
    INF = 1 << 30
    dp = [INF] * (n + 1)
    dp[n] = 0
    nxt = [0] * (n + 1)
    for i in range(n - 1, -1, -1):
        tot = 0
        for j in range(i + 1, min(i + 15, n + 1)):
            tot += ds[j - 1][1]
            if tot > CAP:
                break
            c = (tot + CPC - 1) // CPC * CPC + dp[j]
            if c < dp[i]:
                dp[i] = c
                nxt[i] = j
    segs = []
    i = 0
    while i < n:
        j = nxt[i]
        cur, off = [], 0
        for k in range(i, j):
            cur.append((ds[k][0], off, ds[k][1]))
            off += ds[k][1]
        segs.append((cur, off))
        i = j
    out = []
    col0 = 0
    call0 = 0
    for tiles, ncols_raw in segs:
        ncols = ncols_raw + (-ncols_raw % CPC)
        out.append(dict(tiles=tiles, ncols=ncols, col0=col0, call0=call0,
                        ncalls=ncols // CPC))
        col0 += ncols
        call0 += ncols // CPC
    return dict(D=D, segs=out, ncols=col0, ncalls=call0)


def _host_prep(x, edge_index, edge_weight):
    """Everything host-side: norm, rank layout, ELL columns, idx streams,
    masks, initial narrow table."""
    src = np.asarray(edge_index[0], np.int64)
    dst = np.asarray(edge_index[1], np.int64)
    ew = np.asarray(edge_weight, np.float64)
    xv = np.asarray(x, np.float64).reshape(-1)

    deg = np.bincount(dst, weights=ew, minlength=N)
    dis = np.where(deg > 0, 1.0 / np.sqrt(np.maximum(deg, 1e-30)), 0.0)

    counts = np.bincount(dst, minlength=N)
    order, rank, G = [], [], np.empty(N, np.int64)
    for c in range(NCORE):
        cc = np.zeros(ROWS, np.int64)
        cc[:SHARD] = counts[c * SHARD:(c + 1) * SHARD]
        o = np.argsort(-cc, kind="stable")      # rank i -> padded-local node
        r = np.empty(ROWS, np.int64)
        r[o] = np.arange(ROWS)
        order.append(o)
        rank.append(r)
        G[c * SHARD:(c + 1) * SHARD] = c * ROWS + r[:SHARD]

    gsrc = G[src]
    owner = dst // SHARD

    cnt_all = np.zeros((NCORE, ROWS), np.int64)
    for c in range(NCORE):
        m = owner == c
        gr = rank[c][dst[m] - c * SHARD]
        cnt_all[c] = np.bincount(gr, minlength=ROWS)
    layout = _build_layout(cnt_all)
    segs, NCOLS, NCALLS = layout["segs"], layout["ncols"], layout["ncalls"]

    colbase = np.zeros(TPC, np.int64)
    for s in segs:
        for (t, off, d) in s["tiles"]:
            colbase[t] = s["col0"] + off

    in_maps = []
    for c in range(NCORE):
        m = owner == c
        ls = dst[m] - c * SHARD
        sg = gsrc[m]
        ws = (ew[m] * dis[dst[m]]).astype(np.float64)   # mask = w * dis[dst]
        gr = rank[c][ls]
        oe = np.argsort(gr, kind="stable")
        gr, sg, ws = gr[oe], sg[oe], ws[oe]
        bc = np.bincount(gr, minlength=ROWS)
        starts = np.concatenate([[0], np.cumsum(bc)[:-1]])
        d_within = np.arange(len(gr)) - np.repeat(starts, bc)
        tl = gr // 128
        p = gr % 128
        col = colbase[tl] + d_within

        qidxcol = np.zeros((NCOLS, 128), np.int16)   # wide quad idx
        nidxcol = np.zeros((NCOLS, 128), np.int16)   # narrow oct idx
        wqq = np.zeros((128, NCOLS, 4), np.float32)
        wqn = np.zeros((128, NCOLS, 8), np.float32)
        qidxcol[col, p] = (sg // 4).astype(np.int16)
        nidxcol[col, p] = (sg // 8).astype(np.int16)
        wqq[p, col, sg % 4] = ws
        wqn[p, col, sg % 8] = ws

        def wrap(colmat):
            qc = colmat.reshape(NCALLS, CPC * 128)
            w16 = qc.reshape(NCALLS, IW, 16).transpose(0, 2, 1)
            return np.ascontiguousarray(
                np.tile(w16, (1, 8, 1)).transpose(1, 0, 2).reshape(
                    128, NCALLS * IW))

        qidx = wrap(qidxcol)
        nidx = wrap(nidxcol)

        # wide home uploads
        xs = np.zeros((128, TPC), np.float32)
        dsh = np.zeros((128, TPC), np.float32)
        rmw = np.zeros((128, TPC), np.float32)
        loc = np.minimum(order[c], SHARD - 1)
        vals_x = np.where(order[c] < SHARD, xv[c * SHARD + loc], 0.0)
        vals_d = np.where(order[c] < SHARD, dis[c * SHARD + loc], 0.0)
        i = np.arange(ROWS)
        xs[i % 128, i // 128] = vals_x
        dsh[i % 128, i // 128] = vals_d
        rmw[i % 128, i // 128] = (order[c] < SHARD).astype(np.float32)

        in_maps.append({
            "xsh": xs,
            "dish": dsh,
            "rmask": rmw,
            "qidx": qidx,
            "nidx": nidx,
            "wqq": wqq.astype(bfloat16),
            "wqn": wqn.astype(bfloat16),
        })

    # initial narrow table (x~ = x*dis), same for every core
    nt0 = np.zeros((NT, 8), np.float32)
    nt0[G[np.arange(N)], 0] = (xv * dis).astype(np.float32)
    for c in range(NCORE):
        in_maps[c]["nt0"] = nt0

    return in_maps, layout, order


# ----------------------------------------------------------------------------
def _build_bass(layout):
    import concourse.bass as bass
    import concourse.mybir as mybir
    import concourse.tile as tile
    from concourse import bacc

    F32 = mybir.dt.float32
    BF16 = mybir.dt.bfloat16
    I16 = mybir.dt.int16
    AO = mybir.AluOpType
    AF = mybir.ActivationFunctionType
    AX = mybir.AxisListType

    segs, NCOLS, NCALLS = layout["segs"], layout["ncols"], layout["ncalls"]
    RG = [list(range(NCORE))]

    nc = bacc.Bacc("TRN2", target_bir_lowering=False, debug=False,
                   num_devices=NCORE, num_swdge_queues=4)

    xsh_d = nc.dram_tensor("xsh", [128, TPC], F32, kind="ExternalInput").ap()
    dish_d = nc.dram_tensor("dish", [128, TPC], F32, kind="ExternalInput").ap()
    rmask_d = nc.dram_tensor("rmask", [128, TPC], F32, kind="ExternalInput").ap()
    qidx_d = nc.dram_tensor("qidx", [128, NCALLS * IW], I16, kind="ExternalInput").ap()
    nidx_d = nc.dram_tensor("nidx", [128, NCALLS * IW], I16, kind="ExternalInput").ap()
    wqq_d = nc.dram_tensor("wqq", [128, NCOLS, 4], BF16, kind="ExternalInput").ap()
    wqn_d = nc.dram_tensor("wqn", [128, NCOLS, 8], BF16, kind="ExternalInput").ap()
    nt0_d = nc.dram_tensor("nt0", [NT, 8], F32, kind="ExternalInput").ap()
    coef_d = nc.dram_tensor("coef", [128, 320], F32, kind="ExternalInput").ap()
    wrow_d = nc.dram_tensor("wrow", [128, 768], F32, kind="ExternalInput").ap()
    out_d = nc.dram_tensor("out", [128, TPC], F32, kind="ExternalOutput").ap()

    qrr = [0]

    def next_q():
        q = qrr[0] % 4
        qrr[0] += 1
        return q

    with tile.TileContext(nc) as tc:
        with (
            tc.tile_pool(name="pers", bufs=1) as pp,
            tc.tile_pool(name="qseg", bufs=2) as qp,
            tc.tile_pool(name="stage", bufs=2) as sp,
            tc.tile_pool(name="nmask", bufs=2) as nmp,
            tc.tile_pool(name="tmpw", bufs=1) as tp,
            tc.tile_pool(name="tmpe", bufs=1) as tpe,
            tc.tile_pool(name="psum", bufs=1, space="PSUM") as psp,
            tc.tile_pool(name="dram", bufs=1, space="DRAM") as dp,
        ):
            # persistent tiles
            xsh = pp.tile([128, TPC], F32)
            dish = pp.tile([128, TPC], F32)
            rmask = pp.tile([128, TPC], F32)
            wqq = pp.tile([128, NCOLS, 4], BF16)
            coef = pp.tile([128, 320], F32)
            wrowT = pp.tile([128, 768], F32)
            X = pp.tile([128, TPC, 48], BF16)
            P = pp.tile([128, TPC, 48], F32)
            Z = pp.tile([128, TPC, 48], F32)
            hf = pp.tile([128, TPC, 16], F32)
            tw = pp.tile([128, TPC, 64], BF16)
            ntw = pp.tile([128, TPC, 8], F32)
            nar = pp.tile([128, TPC], F32)
            acc = pp.tile([128, TPC], F32)
            vt = pp.tile([128, TPC, 5], F32)
            sums = pp.tile([128, 32], F32)
            bnst = pp.tile([128, 32], F32)
            s16a = pp.tile([128, 16], F32)
            s16b = pp.tile([128, 16], F32)
            s16c = pp.tile([128, 16], F32)

            tabA = dp.tile([NT, 64], BF16)
            tabB = dp.tile([NT, 64], BF16)
            tabAs = nc.dram_tensor("tabAs", [NT, 64], BF16, kind="Internal",
                                   addr_space="Shared")
            tabBs = nc.dram_tensor("tabBs", [NT, 64], BF16, kind="Internal",
                                   addr_space="Shared")
            tin = dp.tile([ROWS, 64], BF16)
            ntAs = nc.dram_tensor("ntAs", [NT, 8], F32, kind="Internal",
                                  addr_space="Shared")
            ntBs = nc.dram_tensor("ntBs", [NT, 8], F32, kind="Internal",
                                  addr_space="Shared")
            ntin = dp.tile([ROWS, 8], F32)
            bnb1 = dp.tile([1, 32], F32)
            bnb2 = dp.tile([1, 32], F32)

            def cap(i):  # coef scalar AP [128,1]
                return coef[:, i:i + 1]

            # loads
            nc.sync.dma_start(xsh[:], xsh_d[:])
            nc.sync.dma_start(dish[:], dish_d[:])
            nc.sync.dma_start(rmask[:], rmask_d[:])
            nc.sync.dma_start(wqq[:], wqq_d[:])
            nc.sync.dma_start(coef[:], coef_d[:])
            nc.sync.dma_start(wrowT[:], wrow_d[:])

            nc.vector.memset(P[:], 0.0)
            nc.vector.memset(nar[:], 0.0)
            nc.vector.memset(tw[:], 0.0)
            nc.vector.memset(ntw[:], 0.0)

            # ---------------- gather sweeps --------------------------------
            def sweep_wide_fused(tab_ap, tabn_ap):
                build_root_into_Z()
                tabq = tab_ap.rearrange("(q f) c -> q (f c)", f=4)
                tinv = tin[:].rearrange("(t p) c -> p t c", p=128)
                for s in segs:
                    ncalls = s["ncalls"]
                    qs = qp.tile([128, ncalls * IW], I16, tag="qs")
                    nc.sync.dma_start(
                        qs[:], qidx_d[:, s["call0"] * IW:(s["call0"] + ncalls) * IW])
                    st = sp.tile([128, s["ncols"], 256], BF16, tag="st")
                    for ci in range(ncalls):
                        nc.gpsimd.dma_gather(
                            st[:, CPC * ci:CPC * (ci + 1), :], tabq,
                            qs[:, ci * IW:(ci + 1) * IW],
                            CALL, CALL, 256, single_packet=True,
                            queue_num=next_q())
                    nc_s = s["ncols"]
                    c0s = s["col0"]
                    sv = st[:, 0:nc_s, :].rearrange(
                        "p d (q j) -> p d q j", q=4)[:, :, :, 0:48]
                    wv = wqq[:, c0s:c0s + nc_s, :].rearrange(
                        "p d (q u) -> p d q u", u=1).to_broadcast(
                        [128, nc_s, 4, 48])
                    nc.vector.tensor_tensor(sv, sv, wv, AO.mult)
                    tmpq = tp.tile([128, 96, 48], BF16, tag="qred")
                    nc.vector.tensor_tensor(
                        tmpq[:, 0:nc_s, :], st[:, 0:nc_s, 0:48],
                        st[:, 0:nc_s, 64:112], AO.add)
                    nc.vector.tensor_tensor(
                        tmpq[:, 0:nc_s, :], tmpq[:, 0:nc_s, :],
                        st[:, 0:nc_s, 128:176], AO.add)
                    nc.vector.tensor_tensor(
                        tmpq[:, 0:nc_s, :], tmpq[:, 0:nc_s, :],
                        st[:, 0:nc_s, 192:240], AO.add)
                    for (t, off, d) in s["tiles"]:
                        nc.vector.tensor_reduce(
                            P[:, t, :],
                            tmpq[:, off:off + d, :].rearrange("p d j -> p j d"),
                            axis=AX.X, op=AO.add)
                    # fused combine + flush for this segment's tile range
                    t0 = s["tiles"][0][0]
                    t1 = s["tiles"][-1][0] + 1
                    ntl = t1 - t0
                    for k in range(K):
                        tmpE = tpe.tile([128, 14, 16, 16], BF16, tag="tmpE")
                        pb = P[:, t0:t1, k * 16:(k + 1) * 16].rearrange(
                            "p t (u i) -> p t u i", u=1).to_broadcast(
                            [128, ntl, 16, 16])
                        wv2 = wrowT[:, k * 256:(k + 1) * 256].rearrange(
                            "p (u o i) -> p u o i", u=1, o=16).to_broadcast(
                            [128, ntl, 16, 16])
                        nc.vector.tensor_tensor(tmpE[:, 0:ntl], pb, wv2, AO.mult)
                        zt = tpe.tile([128, 14, 16], F32, tag="ztE")
                        nc.vector.tensor_reduce(zt[:, 0:ntl], tmpE[:, 0:ntl],
                                                axis=AX.X, op=AO.add)
                        nc.vector.tensor_tensor(
                            Z[:, t0:t1, k * 16:(k + 1) * 16],
                            Z[:, t0:t1, k * 16:(k + 1) * 16],
                            zt[:, 0:ntl], AO.add)
                    nc.scalar.activation(X[:, t0:t1, :], Z[:, t0:t1, :], AF.Relu)
                    if tabn_ap is not None:
                        nc.vector.tensor_tensor(
                            tw[:, t0:t1, 0:48], X[:, t0:t1, :],
                            dish[:, t0:t1].rearrange(
                                "p (t u) -> p t u", u=1).to_broadcast(
                                [128, ntl, 48]), AO.mult)
                        nc.sync.dma_start(tinv[:, t0:t1, :], tw[:, t0:t1, :])
                if tabn_ap is not None:
                    nc.gpsimd.collective_compute(
                        "AllGather", AO.bypass, ins=[tin[:].opt()],
                        outs=[tabn_ap.opt()], replica_groups=RG)

            def sweep_narrow(ntab_ap):
                ntq = ntab_ap.rearrange("(q f) c -> q (f c)", f=8)
                for s in segs:
                    ncalls = s["ncalls"]
                    qs = qp.tile([128, ncalls * IW], I16, tag="nqs")
                    nc.sync.dma_start(
                        qs[:], nidx_d[:, s["call0"] * IW:(s["call0"] + ncalls) * IW])
                    wn = nmp.tile([128, SEGCAP + CPC, 8], BF16, tag="wn")
                    nc.sync.dma_start(
                        wn[:, 0:s["ncols"], :],
                        wqn_d[:, s["col0"]:s["col0"] + s["ncols"], :])
                    stn = sp.tile([128, s["ncols"], 64], F32, tag="st")
                    for ci in range(ncalls):
                        nc.gpsimd.dma_gather(
                            stn[:, CPC * ci:CPC * (ci + 1), :], ntq,
                            qs[:, ci * IW:(ci + 1) * IW],
                            CALL, CALL, 64, single_packet=True,
                            queue_num=next_q())
                    for (t, off, d) in s["tiles"]:
                        tmpn = tp.tile([128, SEGCAP, 8], F32, tag="tmpn")
                        mv = stn[:, off:off + d, :].rearrange(
                            "p d (o j) -> p d o j", o=8)[:, :, :, 0]
                        nc.vector.tensor_tensor(
                            tmpn[:, 0:d], mv, wn[:, off:off + d, :], AO.mult)
                        nc.vector.tensor_reduce(
                            nar[:, t:t + 1], tmpn[:, 0:d],
                            axis=AX.XY, op=AO.add)

            def table_flush(tab_ap):
                nc.sync.dma_start(
                    tin[:].rearrange("(t p) c -> p t c", p=128), tw[:])
                nc.gpsimd.collective_compute(
                    "AllGather", AO.bypass, ins=[tin[:].opt()],
                    outs=[tab_ap.opt()], replica_groups=RG)

            def ntable_flush(src_ap, nt_ap):
                # ntw[:, :, 0] = src * dis ; flush
                nc.vector.tensor_tensor(
                    ntw[:, :, 0:1],
                    src_ap.rearrange("p (t u) -> p t u", u=1),
                    dish[:].rearrange("p (t u) -> p t u", u=1),
                    AO.mult)
                nc.sync.dma_start(
                    ntin[:].rearrange("(t p) c -> p t c", p=128), ntw[:])
                nc.gpsimd.collective_compute(
                    "AllGather", AO.bypass, ins=[ntin[:].opt()],
                    outs=[nt_ap.opt()], replica_groups=RG)

            def build_root_into_Z():
                # Z[:, :, c] = x * rootw_c + b_c
                for c in range(48):
                    bb = coef[:, 96 + c:97 + c].rearrange(
                        "p (t u) -> p t u", u=1).to_broadcast([128, TPC, 1])
                    nc.vector.scalar_tensor_tensor(
                        Z[:, :, c:c + 1],
                        xsh[:].rearrange("p (t u) -> p t u", u=1),
                        cap(48 + c), bb, AO.mult, AO.add)

            TG = 14  # einsum tile-group

            def conv1_combine():
                # Z = root + P@Wblk ; X = relu(Z)   (P already dis[dst]-scaled)
                build_root_into_Z()
                for g0 in range(0, TPC, TG):
                    tg = min(TG, TPC - g0)
                    for k in range(K):
                        tmpE = tpe.tile([128, TG, 16, 16], BF16, tag="tmpE")
                        pb = P[:, g0:g0 + tg, k * 16:(k + 1) * 16].rearrange(
                            "p t (u i) -> p t u i", u=1).to_broadcast(
                            [128, tg, 16, 16])
                        wv = wrowT[:, k * 256:(k + 1) * 256].rearrange(
                            "p (u o i) -> p u o i", u=1, o=16).to_broadcast(
                            [128, tg, 16, 16])
                        nc.vector.tensor_tensor(tmpE[:, 0:tg], pb, wv, AO.mult)
                        zt = tpe.tile([128, TG, 16], F32, tag="ztE")
                        nc.vector.tensor_reduce(zt[:, 0:tg], tmpE[:, 0:tg],
                                                axis=AX.X, op=AO.add)
                        nc.vector.tensor_tensor(
                            Z[:, g0:g0 + tg, k * 16:(k + 1) * 16],
                            Z[:, g0:g0 + tg, k * 16:(k + 1) * 16],
                            zt[:, 0:tg], AO.add)
                nc.scalar.activation(X[:], Z[:], AF.Relu)

            # ---------------- S1: x~ narrow sweep -> X1 --------------------
            sweep_narrow(nt0_d)
            build_root_into_Z()
            for c in range(48):
                nc.vector.scalar_tensor_tensor(
                    Z[:, :, c:c + 1],
                    nar[:].rearrange("p (t u) -> p t u", u=1),
                    cap(0 + c), Z[:, :, c:c + 1], AO.mult, AO.add)
            nc.scalar.activation(X[:], Z[:], AF.Relu)
            nc.vector.tensor_tensor(
                tw[:, :, 0:48], X[:],
                dish[:].rearrange("p (t u) -> p t u", u=1).to_broadcast(
                    [128, TPC, 48]), AO.mult)
            table_flush(tabAs.ap())

            # ---------------- S2..S4 wide sweeps ---------------------------
            sweep_wide_fused(tabAs.ap(), tabBs.ap())
            sweep_wide_fused(tabBs.ap(), tabAs.ap())
            sweep_wide_fused(tabAs.ap(), None)

            h = X  # reuse X storage for post-BN h (first 16 cols)
            # h1 = mean over stacks
            nc.vector.tensor_tensor(hf[:], X[:, :, 0:16], X[:, :, 16:32], AO.add)
            nc.vector.tensor_tensor(hf[:], hf[:], X[:, :, 32:48], AO.add)
            nc.scalar.activation(hf[:], hf[:], AF.Copy, scale=1.0 / 3.0)

            # ---------------- BatchNorm -----------------------------------
            nc.vector.tensor_tensor(
                hf[:], hf[:],
                rmask[:].rearrange("p (t u) -> p t u", u=1).to_broadcast(
                    [128, TPC, 16]), AO.mult)
            nc.vector.tensor_reduce(
                sums[:, 0:16], hf[:].rearrange("p t f -> p f t"),
                axis=AX.X, op=AO.add)
            nc.scalar.activation(Z[:, :, 0:16], hf[:], AF.Square)
            nc.vector.tensor_reduce(
                sums[:, 16:32], Z[:, :, 0:16].rearrange("p t f -> p f t"),
                axis=AX.X, op=AO.add)
            ones_ps = psp.tile([1, 32], F32)
            nc.tensor.matmul(ones_ps[:], coef[:, 263:264], sums[:],
                             start=True, stop=True)
            bnl = pp.tile([1, 32], F32)
            nc.scalar.activation(bnl[:], ones_ps[:], AF.Copy)
            nc.sync.dma_start(bnb1[:], bnl[:])
            nc.gpsimd.collective_compute(
                "AllReduce", AO.add, ins=[bnb1[:].opt()], outs=[bnb2[:].opt()],
                replica_groups=RG)
            nc.sync.dma_start(bnst[:], bnb2[:].to_broadcast([128, 32]))
            nc.scalar.activation(s16a[:], bnst[:, 0:16], AF.Copy, scale=1.0 / N)
            nc.scalar.activation(s16b[:], bnst[:, 16:32], AF.Copy, scale=1.0 / N)
            musq = pp.tile([128, 16], F32)
            nc.scalar.activation(musq[:], s16a[:], AF.Square)
            nc.vector.tensor_tensor(s16b[:], s16b[:], musq[:], AO.subtract)
            sd = pp.tile([128, 16], F32)
            nc.vector.scalar_tensor_tensor(sd[:], s16b[:], BN_EPS, s16b[:],
                                           AO.add, AO.max)
            nc.scalar.activation(sd[:], sd[:], AF.Sqrt)
            rsd = pp.tile([128, 16], F32)
            nc.vector.reciprocal(rsd[:], sd[:])
            nc.vector.tensor_tensor(s16c[:], rsd[:], coef[:, 144:160], AO.mult)
            shf = pp.tile([128, 16], F32)
            nc.vector.tensor_tensor(shf[:], s16a[:], s16c[:], AO.mult)
            nc.vector.tensor_tensor(shf[:], coef[:, 160:176], shf[:], AO.subtract)
            nc.vector.tensor_tensor(
                hf[:], hf[:],
                s16c[:].rearrange("p (u f) -> p u f", u=1).to_broadcast(
                    [128, TPC, 16]), AO.mult)
            nc.vector.tensor_tensor(
                hf[:], hf[:],
                shf[:].rearrange("p (u f) -> p u f", u=1).to_broadcast(
                    [128, TPC, 16]), AO.add)
            nc.scalar.activation(h[:, :, 0:16], hf[:], AF.Relu)

            # ---------------- conv2 projections ----------------------------
            for j in range(5):
                bb = coef[:, 176 + j:177 + j].rearrange(
                    "p (t u) -> p t u", u=1).to_broadcast([128, TPC, 1])
                nc.vector.scalar_tensor_tensor(
                    vt[:, :, j:j + 1], h[:, :, 0:1],
                    cap(181 + j * 16 + 0), bb, AO.mult, AO.add)
                for i in range(1, 16):
                    nc.vector.scalar_tensor_tensor(
                        vt[:, :, j:j + 1], h[:, :, i:i + 1],
                        cap(181 + j * 16 + i), vt[:, :, j:j + 1],
                        AO.mult, AO.add)

            # ---------------- Horner chain ----------------------------------
            nc.scalar.activation(
                acc[:], vt[:, :, 4:5].rearrange("p t u -> p (t u)"), AF.Copy)
            nts = [ntAs, ntBs]
            for step, j in enumerate((3, 2, 1, 0)):
                nt = nts[step % 2]
                ntable_flush(acc[:], nt.ap())
                sweep_narrow(nt.ap())
                nc.vector.tensor_tensor(
                    acc[:], nar[:],
                    vt[:, :, j:j + 1].rearrange("p t u -> p (t u)"), AO.add)

            # ---------------- final linear + sigmoid ------------------------
            outb = pp.tile([128, TPC], F32)
            nc.scalar.activation(outb[:], acc[:], AF.Sigmoid,
                                 scale=cap(261), bias=cap(262))
            nc.sync.dma_start(out_d[:], outb[:])

    nc.compile()
    return nc


# ----------------------------------------------------------------------------
def kernel(x, edge_index, edge_weight, w1_init, w1_w, w1_root, w1_b,
           bn1_g, bn1_b, w2_init, w2_w, w2_root, w2_b, lin_w, lin_b):
    _install_hookshim()
    x = np.asarray(x, np.float32)
    edge_index = np.asarray(edge_index)
    edge_weight = np.asarray(edge_weight, np.float32)

    in_maps, layout, order = _host_prep(x, edge_index, edge_weight)

    # ---- coefficient packing (host): tiny-weight derived scalars
    w1_init = np.asarray(w1_init, np.float64)
    w1_w_ = np.asarray(w1_w, np.float64)
    w1_root = np.asarray(w1_root, np.float64)
    w1_b_ = np.asarray(w1_b, np.float64)
    w2_init = np.asarray(w2_init, np.float64)
    w2_w_ = np.asarray(w2_w, np.float64)
    w2_root = np.asarray(w2_root, np.float64)
    w2_b_ = np.asarray(w2_b, np.float64)

    coef = np.zeros(320, np.float64)
    coef[0:48] = w1_init[:, 0, :].reshape(-1)
    coef[48:96] = w1_root[:, 0, :].reshape(-1)
    coef[96:144] = w1_b_.reshape(-1)
    coef[144:160] = np.asarray(bn1_g, np.float64)
    coef[160:176] = np.asarray(bn1_b, np.float64)
    wk = w2_w_[:, 0, 0]
    gmat = np.zeros((5, 16), np.float64)
    beta = np.zeros(5, np.float64)
    gmat[4] = (wk ** 3 / 3.0) @ w2_init[:, :, 0]
    gmat[3] = (wk ** 3 / 3.0) @ w2_root[:, :, 0]; beta[3] = (wk ** 3 / 3.0) @ w2_b_[:, 0]
    gmat[2] = (wk ** 2 / 3.0) @ w2_root[:, :, 0]; beta[2] = (wk ** 2 / 3.0) @ w2_b_[:, 0]
    gmat[1] = (wk / 3.0) @ w2_root[:, :, 0];      beta[1] = (wk / 3.0) @ w2_b_[:, 0]
    gmat[0] = np.ones(3) / 3.0 @ w2_root[:, :, 0]; beta[0] = np.ones(3) / 3.0 @ w2_b_[:, 0]
    coef[176:181] = beta
    coef[181:261] = gmat.reshape(-1)
    coef[261] = np.asarray(lin_w, np.float64)[0, 0]
    coef[262] = np.asarray(lin_b, np.float64)[0]
    coef[263] = 1.0
    coef_np = np.tile(coef.astype(np.float32)[None, :], (128, 1))

    wrow = np.zeros(768, np.float64)
    for k in range(K):
        for o in range(16):
            wrow[(k * 16 + o) * 16:(k * 16 + o) * 16 + 16] = w1_w_[k, :, o]
    wrow_np = np.tile(wrow.astype(np.float32)[None, :], (128, 1))

    for m in in_maps:
        m["coef"] = coef_np
        m["wrow"] = wrow_np

    nc = _build_bass(layout)
    from concourse.bass_utils import run_bass_kernel_spmd
    trace = os.environ.get("BASS_GNN_TRACE", "0") == "1"
    res = run_bass_kernel_spmd(nc, in_maps, core_ids=list(range(NCORE)),
                               trace=trace)
    _EXEC_NS[0] = res.exec_time_ns

    out = np.empty((N, 1), np.float32)
    for c in range(NCORE):
        ob = res.results[c]["out"]        # [128, TPC]
        i = np.arange(ROWS)
        vals = ob[i % 128, i // 128]       # value at rank i
        keep = order[c] < SHARD
        out[c * SHARD + order[c][keep], 0] = vals[keep]
    return out


def last_exec_ns():
    return _EXEC_NS[0]


# revision 16
# speedup vs baseline: 1.1303x; 1.1303x over previous
"""Trainium2 Bass kernel for nn_ArmaNet02 (ARMA GNN, N=100K, E=1.6M, K=3, T=4, H=16).

Strategy (8 NeuronCores, SPMD), v2:
- dst-sharded ELL, wide gathers use bf16 QUADS (4 nodes x 64 cols = 512B,
  int16 quad indices) from the replicated feature table [NT, 64].
- Narrow state (conv1 t=0 input and the conv2 Horner chain) lives in a
  dedicated compact table [NT, 8] fp32; gather elements are OCTs (8 nodes x
  32B = 256B, single int16 index space).  Horner flushes/AllGathers move
  3.2MB instead of 12.8MB, and the initial x~ table is host-provided so the
  first flush+AllGather disappears.
- gcn_norm folded: dis[src] into table writes, dis[dst] into the edge masks
  (dis computed on host; no on-device degree phase).
"""

import os
import sys
import types

import numpy as np
from ml_dtypes import bfloat16

# ----------------------------------------------------------------------------
# problem constants (hardcoded; kernel.py must be self-contained)
N = 100000
E = 1600000
K = 3
T = 4
H = 16
BN_EPS = 1e-5
NCORE = 8
SHARD = 12500
TPC = 98                 # tiles per core (12544 rows)
ROWS = TPC * 128         # 12544
NT = NCORE * ROWS        # 100352 table rows
NQUAD = NT // 4          # 25088 quad elements (wide)
NOCT = NT // 8           # 12544 oct elements (narrow table)
CPC = 8                  # columns per gather call (8*128 = 1024 slots)
CALL = CPC * 128
IW = CALL // 16          # idx words per call per partition
SEGCAP = 88              # max data columns per segment (pre-padding)

_EXEC_NS = [None]


def _install_hookshim():
    if "antenv.axon_hooks" in sys.modules:
        return
    try:
        import antenv
    except ImportError:
        return
    mod = types.ModuleType("antenv.axon_hooks")
    mod._hook = None
    mod.set_axon_ntff_profile_hook = lambda h: setattr(mod, "_hook", h)
    mod.get_axon_ntff_profile_hook = lambda: mod._hook
    sys.modules["antenv.axon_hooks"] = mod
    antenv.axon_hooks = mod
    try:
        from trn_agent_boot.trn_boot import _ntff_profile_via_ctypes
        hook = _ntff_profile_via_ctypes("/opt/axon/libaxon_pjrt.so")
        if hook is not None:
            mod.set_axon_ntff_profile_hook(hook)
    except Exception:
        pass


# ----------------------------------------------------------------------------
def _build_layout(cnt_all):
    """cnt_all: [8, ROWS] per-dst edge counts in rank order.  Shared layout."""
    D = np.zeros(TPC, np.int64)
    for c in range(NCORE):
        for t in range(TPC):
            D[t] = max(D[t], cnt_all[c][t * 128:(t + 1) * 128].max(initial=0))
    ds = [(t, int(D[t])) for t in range(TPC) if D[t] > 0]
    n = len(ds)
    CAP = SEGCAP
    INF = 1 << 30
    dp = [INF] * (n + 1)
    dp[n] = 0
    nxt = [0] * (n + 1)
    for i in range(n - 1, -1, -1):
        tot = 0
        for j in range(i + 1, min(i + 15, n + 1)):
            tot += ds[j - 1][1]
            if tot > CAP:
                break
            c = (tot + CPC - 1) // CPC * CPC + CPC + dp[j]
            if c < dp[i]:
                dp[i] = c
                nxt[i] = j
    segs = []
    i = 0
    while i < n:
        j = nxt[i]
        cur, off = [], 0
        for k in range(i, j):
            cur.append((ds[k][0], off, ds[k][1]))
            off += ds[k][1]
        segs.append((cur, off))
        i = j
    out = []
    col0 = 0
    call0 = 0
    for tiles, ncols_raw in segs:
        ncols = ncols_raw + (-ncols_raw % CPC)
        out.append(dict(tiles=tiles, ncols=ncols, col0=col0, call0=call0,
                        ncalls=ncols // CPC))
        col0 += ncols
        call0 += ncols // CPC
    return dict(D=D, segs=out, ncols=col0, ncalls=call0)


def _host_prep(x, edge_index, edge_weight):
    """Everything host-side: norm, rank layout, ELL columns, idx streams,
    masks, initial narrow table."""
    src = np.asarray(edge_index[0], np.int64)
    dst = np.asarray(edge_index[1], np.int64)
    ew = np.asarray(edge_weight, np.float64)
    xv = np.asarray(x, np.float64).reshape(-1)

    deg = np.bincount(dst, weights=ew, minlength=N)
    dis = np.where(deg > 0, 1.0 / np.sqrt(np.maximum(deg, 1e-30)), 0.0)

    counts = np.bincount(dst, minlength=N)
    order, rank, G = [], [], np.empty(N, np.int64)
    for c in range(NCORE):
        cc = np.zeros(ROWS, np.int64)
        cc[:SHARD] = counts[c * SHARD:(c + 1) * SHARD]
        o = np.argsort(-cc, kind="stable")      # rank i -> padded-local node
        r = np.empty(ROWS, np.int64)
        r[o] = np.arange(ROWS)
        order.append(o)
        rank.append(r)
        G[c * SHARD:(c + 1) * SHARD] = c * ROWS + r[:SHARD]

    gsrc = G[src]
    owner = dst // SHARD

    cnt_all = np.zeros((NCORE, ROWS), np.int64)
    for c in range(NCORE):
        m = owner == c
        gr = rank[c][dst[m] - c * SHARD]
        cnt_all[c] = np.bincount(gr, minlength=ROWS)
    layout = _build_layout(cnt_all)
    segs, NCOLS, NCALLS = layout["segs"], layout["ncols"], layout["ncalls"]

    colbase = np.zeros(TPC, np.int64)
    for s in segs:
        for (t, off, d) in s["tiles"]:
            colbase[t] = s["col0"] + off

    in_maps = []
    for c in range(NCORE):
        m = owner == c
        ls = dst[m] - c * SHARD
        sg = gsrc[m]
        ws = (ew[m] * dis[dst[m]]).astype(np.float64)   # mask = w * dis[dst]
        gr = rank[c][ls]
        oe = np.argsort(gr, kind="stable")
        gr, sg, ws = gr[oe], sg[oe], ws[oe]
        bc = np.bincount(gr, minlength=ROWS)
        starts = np.concatenate([[0], np.cumsum(bc)[:-1]])
        d_within = np.arange(len(gr)) - np.repeat(starts, bc)
        tl = gr // 128
        p = gr % 128
        col = colbase[tl] + d_within

        qidxcol = np.zeros((NCOLS, 128), np.int16)   # wide quad idx
        nidxcol = np.zeros((NCOLS, 128), np.int16)   # narrow oct idx
        wqq = np.zeros((128, NCOLS, 4), np.float32)
        wqn = np.zeros((128, NCOLS, 8), np.float32)
        qidxcol[col, p] = (sg // 4).astype(np.int16)
        nidxcol[col, p] = (sg // 8).astype(np.int16)
        wqq[p, col, sg % 4] = ws
        wqn[p, col, sg % 8] = ws

        def wrap(colmat):
            qc = colmat.reshape(NCALLS, CPC * 128)
            w16 = qc.reshape(NCALLS, IW, 16).transpose(0, 2, 1)
            return np.ascontiguousarray(
                np.tile(w16, (1, 8, 1)).transpose(1, 0, 2).reshape(
                    128, NCALLS * IW))

        qidx = wrap(qidxcol)
        nidx = wrap(nidxcol)

        # wide home uploads
        xs = np.zeros((128, TPC), np.float32)
        dsh = np.zeros((128, TPC), np.float32)
        rmw = np.zeros((128, TPC), np.float32)
        loc = np.minimum(order[c], SHARD - 1)
        vals_x = np.where(order[c] < SHARD, xv[c * SHARD + loc], 0.0)
        vals_d = np.where(order[c] < SHARD, dis[c * SHARD + loc], 0.0)
        i = np.arange(ROWS)
        xs[i % 128, i // 128] = vals_x
        dsh[i % 128, i // 128] = vals_d
        rmw[i % 128, i // 128] = (order[c] < SHARD).astype(np.float32)

        in_maps.append({
            "xsh": xs,
            "dish": dsh,
            "rmask": rmw,
            "qidx": qidx,
            "nidx": nidx,
            "wqq": wqq.astype(bfloat16),
            "wqn": wqn.astype(bfloat16),
        })

    # initial narrow table (x~ = x*dis), same for every core
    nt0 = np.zeros((NT, 8), np.float32)
    nt0[G[np.arange(N)], 0] = (xv * dis).astype(np.float32)
    for c in range(NCORE):
        in_maps[c]["nt0"] = nt0

    return in_maps, layout, order


# ----------------------------------------------------------------------------
def _build_bass(layout):
    import concourse.bass as bass
    import concourse.mybir as mybir
    import concourse.tile as tile
    from concourse import bacc

    F32 = mybir.dt.float32
    BF16 = mybir.dt.bfloat16
    I16 = mybir.dt.int16
    AO = mybir.AluOpType
    AF = mybir.ActivationFunctionType
    AX = mybir.AxisListType

    segs, NCOLS, NCALLS = layout["segs"], layout["ncols"], layout["ncalls"]
    RG = [list(range(NCORE))]

    nc = bacc.Bacc("TRN2", target_bir_lowering=False, debug=False,
                   num_devices=NCORE, num_swdge_queues=4)

    xsh_d = nc.dram_tensor("xsh", [128, TPC], F32, kind="ExternalInput").ap()
    dish_d = nc.dram_tensor("dish", [128, TPC], F32, kind="ExternalInput").ap()
    rmask_d = nc.dram_tensor("rmask", [128, TPC], F32, kind="ExternalInput").ap()
    qidx_d = nc.dram_tensor("qidx", [128, NCALLS * IW], I16, kind="ExternalInput").ap()
    nidx_d = nc.dram_tensor("nidx", [128, NCALLS * IW], I16, kind="ExternalInput").ap()
    wqq_d = nc.dram_tensor("wqq", [128, NCOLS, 4], BF16, kind="ExternalInput").ap()
    wqn_d = nc.dram_tensor("wqn", [128, NCOLS, 8], BF16, kind="ExternalInput").ap()
    nt0_d = nc.dram_tensor("nt0", [NT, 8], F32, kind="ExternalInput").ap()
    coef_d = nc.dram_tensor("coef", [128, 320], F32, kind="ExternalInput").ap()
    wrow_d = nc.dram_tensor("wrow", [128, 768], F32, kind="ExternalInput").ap()
    out_d = nc.dram_tensor("out", [128, TPC], F32, kind="ExternalOutput").ap()

    qrr = [0]

    def next_q():
        q = qrr[0] % 4
        qrr[0] += 1
        return q

    with tile.TileContext(nc) as tc:
        with (
            tc.tile_pool(name="pers", bufs=1) as pp,
            tc.tile_pool(name="qseg", bufs=2) as qp,
            tc.tile_pool(name="stage", bufs=2) as sp,
            tc.tile_pool(name="nmask", bufs=2) as nmp,
            tc.tile_pool(name="tmpw", bufs=1) as tp,
            tc.tile_pool(name="tmpe", bufs=1) as tpe,
            tc.tile_pool(name="psum", bufs=1, space="PSUM") as psp,
            tc.tile_pool(name="dram", bufs=1, space="DRAM") as dp,
        ):
            # persistent tiles
            xsh = pp.tile([128, TPC], F32)
            dish = pp.tile([128, TPC], F32)
            rmask = pp.tile([128, TPC], F32)
            wqq = pp.tile([128, NCOLS, 4], BF16)
            coef = pp.tile([128, 320], F32)
            wrowT = pp.tile([128, 768], F32)
            X = pp.tile([128, TPC, 48], BF16)
            P = pp.tile([128, TPC, 48], F32)
            Z = pp.tile([128, TPC, 48], F32)
            hf = pp.tile([128, TPC, 16], F32)
            tw = pp.tile([128, TPC, 64], BF16)
            ntw = pp.tile([128, TPC, 8], F32)
            nar = pp.tile([128, TPC], F32)
            acc = pp.tile([128, TPC], F32)
            vt = pp.tile([128, TPC, 5], F32)
            sums = pp.tile([128, 32], F32)
            bnst = pp.tile([128, 32], F32)
            s16a = pp.tile([128, 16], F32)
            s16b = pp.tile([128, 16], F32)
            s16c = pp.tile([128, 16], F32)

            tabA = dp.tile([NT, 64], BF16)
            tabB = dp.tile([NT, 64], BF16)
            tabAs = nc.dram_tensor("tabAs", [NT, 64], BF16, kind="Internal",
                                   addr_space="Shared")
            tabBs = nc.dram_tensor("tabBs", [NT, 64], BF16, kind="Internal",
                                   addr_space="Shared")
            tin = dp.tile([ROWS, 64], BF16)
            ntAs = nc.dram_tensor("ntAs", [NT, 8], F32, kind="Internal",
                                  addr_space="Shared")
            ntBs = nc.dram_tensor("ntBs", [NT, 8], F32, kind="Internal",
                                  addr_space="Shared")
            ntin = dp.tile([ROWS, 8], F32)
            bnb1 = dp.tile([1, 32], F32)
            bnb2 = dp.tile([1, 32], F32)

            def cap(i):  # coef scalar AP [128,1]
                return coef[:, i:i + 1]

            # loads
            nc.sync.dma_start(xsh[:], xsh_d[:])
            nc.sync.dma_start(dish[:], dish_d[:])
            nc.sync.dma_start(rmask[:], rmask_d[:])
            nc.sync.dma_start(wqq[:], wqq_d[:])
            nc.sync.dma_start(coef[:], coef_d[:])
            nc.sync.dma_start(wrowT[:], wrow_d[:])

            nc.vector.memset(P[:], 0.0)
            nc.vector.memset(nar[:], 0.0)
            nc.vector.memset(tw[:], 0.0)
            nc.vector.memset(ntw[:], 0.0)

            # ---------------- gather sweeps --------------------------------
            def sweep_wide_fused(tab_ap, tabn_ap):
                build_root_into_Z()
                tabq = tab_ap.rearrange("(q f) c -> q (f c)", f=4)
                tinv = tin[:].rearrange("(t p) c -> p t c", p=128)
                for s in segs:
                    ncalls = s["ncalls"]
                    qs = qp.tile([128, ncalls * IW], I16, tag="qs")
                    nc.sync.dma_start(
                        qs[:], qidx_d[:, s["call0"] * IW:(s["call0"] + ncalls) * IW])
                    st = sp.tile([128, s["ncols"], 256], BF16, tag="st")
                    for ci in range(ncalls):
                        nc.gpsimd.dma_gather(
                            st[:, CPC * ci:CPC * (ci + 1), :], tabq,
                            qs[:, ci * IW:(ci + 1) * IW],
                            CALL, CALL, 256, single_packet=True,
                            queue_num=next_q())
                    nc_s = s["ncols"]
                    c0s = s["col0"]
                    sv = st[:, 0:nc_s, :].rearrange(
                        "p d (q j) -> p d q j", q=4)[:, :, :, 0:48]
                    wv = wqq[:, c0s:c0s + nc_s, :].rearrange(
                        "p d (q u) -> p d q u", u=1).to_broadcast(
                        [128, nc_s, 4, 48])
                    nc.vector.tensor_tensor(sv, sv, wv, AO.mult)
                    tmpq = tp.tile([128, 96, 48], BF16, tag="qred")
                    nc.vector.tensor_tensor(
                        tmpq[:, 0:nc_s, :], st[:, 0:nc_s, 0:48],
                        st[:, 0:nc_s, 64:112], AO.add)
                    nc.vector.tensor_tensor(
                        tmpq[:, 0:nc_s, :], tmpq[:, 0:nc_s, :],
                        st[:, 0:nc_s, 128:176], AO.add)
                    nc.vector.tensor_tensor(
                        tmpq[:, 0:nc_s, :], tmpq[:, 0:nc_s, :],
                        st[:, 0:nc_s, 192:240], AO.add)
                    for (t, off, d) in s["tiles"]:
                        nc.vector.tensor_reduce(
                            P[:, t, :],
                            tmpq[:, off:off + d, :].rearrange("p d j -> p j d"),
                            axis=AX.X, op=AO.add)
                    # fused combine + flush for this segment's tile range
                    t0 = s["tiles"][0][0]
                    t1 = s["tiles"][-1][0] + 1
                    ntl = t1 - t0
                    for k in range(K):
                        tmpE = tpe.tile([128, 14, 16, 16], BF16, tag="tmpE")
                        pb = P[:, t0:t1, k * 16:(k + 1) * 16].rearrange(
                            "p t (u i) -> p t u i", u=1).to_broadcast(
                            [128, ntl, 16, 16])
                        wv2 = wrowT[:, k * 256:(k + 1) * 256].rearrange(
                            "p (u o i) -> p u o i", u=1, o=16).to_broadcast(
                            [128, ntl, 16, 16])
                        nc.vector.tensor_tensor(tmpE[:, 0:ntl], pb, wv2, AO.mult)
                        zt = tpe.tile([128, 14, 16], F32, tag="ztE")
                        nc.vector.tensor_reduce(zt[:, 0:ntl], tmpE[:, 0:ntl],
                                                axis=AX.X, op=AO.add)
                        nc.vector.tensor_tensor(
                            Z[:, t0:t1, k * 16:(k + 1) * 16],
                            Z[:, t0:t1, k * 16:(k + 1) * 16],
                            zt[:, 0:ntl], AO.add)
                    nc.scalar.activation(X[:, t0:t1, :], Z[:, t0:t1, :], AF.Relu)
                    if tabn_ap is not None:
                        nc.vector.tensor_tensor(
                            tw[:, t0:t1, 0:48], X[:, t0:t1, :],
                            dish[:, t0:t1].rearrange(
                                "p (t u) -> p t u", u=1).to_broadcast(
                                [128, ntl, 48]), AO.mult)
                        nc.sync.dma_start(tinv[:, t0:t1, :], tw[:, t0:t1, :])
                if tabn_ap is not None:
                    nc.gpsimd.collective_compute(
                        "AllGather", AO.bypass, ins=[tin[:].opt()],
                        outs=[tabn_ap.opt()], replica_groups=RG)

            def sweep_narrow(ntab_ap):
                ntq = ntab_ap.rearrange("(q f) c -> q (f c)", f=8)
                for s in segs:
                    ncalls = s["ncalls"]
                    qs = qp.tile([128, ncalls * IW], I16, tag="nqs")
                    nc.sync.dma_start(
                        qs[:], nidx_d[:, s["call0"] * IW:(s["call0"] + ncalls) * IW])
                    wn = nmp.tile([128, SEGCAP + CPC, 8], BF16, tag="wn")
                    nc.sync.dma_start(
                        wn[:, 0:s["ncols"], :],
                        wqn_d[:, s["col0"]:s["col0"] + s["ncols"], :])
                    stn = sp.tile([128, s["ncols"], 64], F32, tag="st")
                    for ci in range(ncalls):
                        nc.gpsimd.dma_gather(
                            stn[:, CPC * ci:CPC * (ci + 1), :], ntq,
                            qs[:, ci * IW:(ci + 1) * IW],
                            CALL, CALL, 64, single_packet=True,
                            queue_num=next_q())
                    for (t, off, d) in s["tiles"]:
                        tmpn = tp.tile([128, SEGCAP, 8], F32, tag="tmpn")
                        mv = stn[:, off:off + d, :].rearrange(
                            "p d (o j) -> p d o j", o=8)[:, :, :, 0]
                        nc.vector.tensor_tensor(
                            tmpn[:, 0:d], mv, wn[:, off:off + d, :], AO.mult)
                        nc.vector.tensor_reduce(
                            nar[:, t:t + 1], tmpn[:, 0:d],
                            axis=AX.XY, op=AO.add)

            def table_flush(tab_ap):
                nc.sync.dma_start(
                    tin[:].rearrange("(t p) c -> p t c", p=128), tw[:])
                nc.gpsimd.collective_compute(
                    "AllGather", AO.bypass, ins=[tin[:].opt()],
                    outs=[tab_ap.opt()], replica_groups=RG)

            def ntable_flush(src_ap, nt_ap):
                # ntw[:, :, 0] = src * dis ; flush
                nc.vector.tensor_tensor(
                    ntw[:, :, 0:1],
                    src_ap.rearrange("p (t u) -> p t u", u=1),
                    dish[:].rearrange("p (t u) -> p t u", u=1),
                    AO.mult)
                nc.sync.dma_start(
                    ntin[:].rearrange("(t p) c -> p t c", p=128), ntw[:])
                nc.gpsimd.collective_compute(
                    "AllGather", AO.bypass, ins=[ntin[:].opt()],
                    outs=[nt_ap.opt()], replica_groups=RG)

            def build_root_into_Z():
                # Z[:, :, c] = x * rootw_c + b_c
                for c in range(48):
                    bb = coef[:, 96 + c:97 + c].rearrange(
                        "p (t u) -> p t u", u=1).to_broadcast([128, TPC, 1])
                    nc.vector.scalar_tensor_tensor(
                        Z[:, :, c:c + 1],
                        xsh[:].rearrange("p (t u) -> p t u", u=1),
                        cap(48 + c), bb, AO.mult, AO.add)

            TG = 14  # einsum tile-group

            def conv1_combine():
                # Z = root + P@Wblk ; X = relu(Z)   (P already dis[dst]-scaled)
                build_root_into_Z()
                for g0 in range(0, TPC, TG):
                    tg = min(TG, TPC - g0)
                    for k in range(K):
                        tmpE = tpe.tile([128, TG, 16, 16], BF16, tag="tmpE")
                        pb = P[:, g0:g0 + tg, k * 16:(k + 1) * 16].rearrange(
                            "p t (u i) -> p t u i", u=1).to_broadcast(
                            [128, tg, 16, 16])
                        wv = wrowT[:, k * 256:(k + 1) * 256].rearrange(
                            "p (u o i) -> p u o i", u=1, o=16).to_broadcast(
                            [128, tg, 16, 16])
                        nc.vector.tensor_tensor(tmpE[:, 0:tg], pb, wv, AO.mult)
                        zt = tpe.tile([128, TG, 16], F32, tag="ztE")
                        nc.vector.tensor_reduce(zt[:, 0:tg], tmpE[:, 0:tg],
                                                axis=AX.X, op=AO.add)
                        nc.vector.tensor_tensor(
                            Z[:, g0:g0 + tg, k * 16:(k + 1) * 16],
                            Z[:, g0:g0 + tg, k * 16:(k + 1) * 16],
                            zt[:, 0:tg], AO.add)
                nc.scalar.activation(X[:], Z[:], AF.Relu)

            # ---------------- S1: x~ narrow sweep -> X1 --------------------
            sweep_narrow(nt0_d)
            build_root_into_Z()
            for c in range(48):
                nc.vector.scalar_tensor_tensor(
                    Z[:, :, c:c + 1],
                    nar[:].rearrange("p (t u) -> p t u", u=1),
                    cap(0 + c), Z[:, :, c:c + 1], AO.mult, AO.add)
            nc.scalar.activation(X[:], Z[:], AF.Relu)
            nc.vector.tensor_tensor(
                tw[:, :, 0:48], X[:],
                dish[:].rearrange("p (t u) -> p t u", u=1).to_broadcast(
                    [128, TPC, 48]), AO.mult)
            table_flush(tabAs.ap())

            # ---------------- S2..S4 wide sweeps ---------------------------
            sweep_wide_fused(tabAs.ap(), tabBs.ap())
            sweep_wide_fused(tabBs.ap(), tabAs.ap())
            sweep_wide_fused(tabAs.ap(), None)

            h = X  # reuse X storage for post-BN h (first 16 cols)
            # h1 = mean over stacks
            nc.vector.tensor_tensor(hf[:], X[:, :, 0:16], X[:, :, 16:32], AO.add)
            nc.vector.tensor_tensor(hf[:], hf[:], X[:, :, 32:48], AO.add)
            nc.scalar.activation(hf[:], hf[:], AF.Copy, scale=1.0 / 3.0)

            # ---------------- BatchNorm -----------------------------------
            nc.vector.tensor_tensor(
                hf[:], hf[:],
                rmask[:].rearrange("p (t u) -> p t u", u=1).to_broadcast(
                    [128, TPC, 16]), AO.mult)
            nc.vector.tensor_reduce(
                sums[:, 0:16], hf[:].rearrange("p t f -> p f t"),
                axis=AX.X, op=AO.add)
            nc.scalar.activation(Z[:, :, 0:16], hf[:], AF.Square)
            nc.vector.tensor_reduce(
                sums[:, 16:32], Z[:, :, 0:16].rearrange("p t f -> p f t"),
                axis=AX.X, op=AO.add)
            ones_ps = psp.tile([1, 32], F32)
            nc.tensor.matmul(ones_ps[:], coef[:, 263:264], sums[:],
                             start=True, stop=True)
            bnl = pp.tile([1, 32], F32)
            nc.scalar.activation(bnl[:], ones_ps[:], AF.Copy)
            nc.sync.dma_start(bnb1[:], bnl[:])
            nc.gpsimd.collective_compute(
                "AllReduce", AO.add, ins=[bnb1[:].opt()], outs=[bnb2[:].opt()],
                replica_groups=RG)
            nc.sync.dma_start(bnst[:], bnb2[:].to_broadcast([128, 32]))
            nc.scalar.activation(s16a[:], bnst[:, 0:16], AF.Copy, scale=1.0 / N)
            nc.scalar.activation(s16b[:], bnst[:, 16:32], AF.Copy, scale=1.0 / N)
            musq = pp.tile([128, 16], F32)
            nc.scalar.activation(musq[:], s16a[:], AF.Square)
            nc.vector.tensor_tensor(s16b[:], s16b[:], musq[:], AO.subtract)
            sd = pp.tile([128, 16], F32)
            nc.vector.scalar_tensor_tensor(sd[:], s16b[:], BN_EPS, s16b[:],
                                           AO.add, AO.max)
            nc.scalar.activation(sd[:], sd[:], AF.Sqrt)
            rsd = pp.tile([128, 16], F32)
            nc.vector.reciprocal(rsd[:], sd[:])
            nc.vector.tensor_tensor(s16c[:], rsd[:], coef[:, 144:160], AO.mult)
            shf = pp.tile([128, 16], F32)
            nc.vector.tensor_tensor(shf[:], s16a[:], s16c[:], AO.mult)
            nc.vector.tensor_tensor(shf[:], coef[:, 160:176], shf[:], AO.subtract)
            nc.vector.tensor_tensor(
                hf[:], hf[:],
                s16c[:].rearrange("p (u f) -> p u f", u=1).to_broadcast(
                    [128, TPC, 16]), AO.mult)
            nc.vector.tensor_tensor(
                hf[:], hf[:],
                shf[:].rearrange("p (u f) -> p u f", u=1).to_broadcast(
                    [128, TPC, 16]), AO.add)
            nc.scalar.activation(h[:, :, 0:16], hf[:], AF.Relu)

            # ---------------- conv2 projections ----------------------------
            for j in range(5):
                bb = coef[:, 176 + j:177 + j].rearrange(
                    "p (t u) -> p t u", u=1).to_broadcast([128, TPC, 1])
                nc.vector.scalar_tensor_tensor(
                    vt[:, :, j:j + 1], h[:, :, 0:1],
                    cap(181 + j * 16 + 0), bb, AO.mult, AO.add)
                for i in range(1, 16):
                    nc.vector.scalar_tensor_tensor(
                        vt[:, :, j:j + 1], h[:, :, i:i + 1],
                        cap(181 + j * 16 + i), vt[:, :, j:j + 1],
                        AO.mult, AO.add)

            # ---------------- Horner chain ----------------------------------
            nc.scalar.activation(
                acc[:], vt[:, :, 4:5].rearrange("p t u -> p (t u)"), AF.Copy)
            nts = [ntAs, ntBs]
            for step, j in enumerate((3, 2, 1, 0)):
                nt = nts[step % 2]
                ntable_flush(acc[:], nt.ap())
                sweep_narrow(nt.ap())
                nc.vector.tensor_tensor(
                    acc[:], nar[:],
                    vt[:, :, j:j + 1].rearrange("p t u -> p (t u)"), AO.add)

            # ---------------- final linear + sigmoid ------------------------
            outb = pp.tile([128, TPC], F32)
            nc.scalar.activation(outb[:], acc[:], AF.Sigmoid,
                                 scale=cap(261), bias=cap(262))
            nc.sync.dma_start(out_d[:], outb[:])

    nc.compile()
    return nc


# ----------------------------------------------------------------------------
def kernel(x, edge_index, edge_weight, w1_init, w1_w, w1_root, w1_b,
           bn1_g, bn1_b, w2_init, w2_w, w2_root, w2_b, lin_w, lin_b):
    _install_hookshim()
    x = np.asarray(x, np.float32)
    edge_index = np.asarray(edge_index)
    edge_weight = np.asarray(edge_weight, np.float32)

    in_maps, layout, order = _host_prep(x, edge_index, edge_weight)

    # ---- coefficient packing (host): tiny-weight derived scalars
    w1_init = np.asarray(w1_init, np.float64)
    w1_w_ = np.asarray(w1_w, np.float64)
    w1_root = np.asarray(w1_root, np.float64)
    w1_b_ = np.asarray(w1_b, np.float64)
    w2_init = np.asarray(w2_init, np.float64)
    w2_w_ = np.asarray(w2_w, np.float64)
    w2_root = np.asarray(w2_root, np.float64)
    w2_b_ = np.asarray(w2_b, np.float64)

    coef = np.zeros(320, np.float64)
    coef[0:48] = w1_init[:, 0, :].reshape(-1)
    coef[48:96] = w1_root[:, 0, :].reshape(-1)
    coef[96:144] = w1_b_.reshape(-1)
    coef[144:160] = np.asarray(bn1_g, np.float64)
    coef[160:176] = np.asarray(bn1_b, np.float64)
    wk = w2_w_[:, 0, 0]
    gmat = np.zeros((5, 16), np.float64)
    beta = np.zeros(5, np.float64)
    gmat[4] = (wk ** 3 / 3.0) @ w2_init[:, :, 0]
    gmat[3] = (wk ** 3 / 3.0) @ w2_root[:, :, 0]; beta[3] = (wk ** 3 / 3.0) @ w2_b_[:, 0]
    gmat[2] = (wk ** 2 / 3.0) @ w2_root[:, :, 0]; beta[2] = (wk ** 2 / 3.0) @ w2_b_[:, 0]
    gmat[1] = (wk / 3.0) @ w2_root[:, :, 0];      beta[1] = (wk / 3.0) @ w2_b_[:, 0]
    gmat[0] = np.ones(3) / 3.0 @ w2_root[:, :, 0]; beta[0] = np.ones(3) / 3.0 @ w2_b_[:, 0]
    coef[176:181] = beta
    coef[181:261] = gmat.reshape(-1)
    coef[261] = np.asarray(lin_w, np.float64)[0, 0]
    coef[262] = np.asarray(lin_b, np.float64)[0]
    coef[263] = 1.0
    coef_np = np.tile(coef.astype(np.float32)[None, :], (128, 1))

    wrow = np.zeros(768, np.float64)
    for k in range(K):
        for o in range(16):
            wrow[(k * 16 + o) * 16:(k * 16 + o) * 16 + 16] = w1_w_[k, :, o]
    wrow_np = np.tile(wrow.astype(np.float32)[None, :], (128, 1))

    for m in in_maps:
        m["coef"] = coef_np
        m["wrow"] = wrow_np

    nc = _build_bass(layout)
    from concourse.bass_utils import run_bass_kernel_spmd
    trace = os.environ.get("BASS_GNN_TRACE", "0") == "1"
    res = run_bass_kernel_spmd(nc, in_maps, core_ids=list(range(NCORE)),
                               trace=trace)
    _EXEC_NS[0] = res.exec_time_ns

    out = np.empty((N, 1), np.float32)
    for c in range(NCORE):
        ob = res.results[c]["out"]        # [128, TPC]
        i = np.arange(ROWS)
        vals = ob[i % 128, i // 128]       # value at rank i
        keep = order[c] < SHARD
        out[c * SHARD + order[c][keep], 0] = vals[keep]
    return out


def last_exec_ns():
    return _EXEC_NS[0]


# revision 17
# speedup vs baseline: 1.2275x; 1.0860x over previous
"""Trainium2 Bass kernel for nn_ArmaNet02 (ARMA GNN, N=100K, E=1.6M, K=3, T=4, H=16).

Strategy (8 NeuronCores, SPMD), v2:
- dst-sharded ELL, wide gathers use bf16 QUADS (4 nodes x 64 cols = 512B,
  int16 quad indices) from the replicated feature table [NT, 64].
- Narrow state (conv1 t=0 input and the conv2 Horner chain) lives in a
  dedicated compact table [NT, 8] fp32; gather elements are OCTs (8 nodes x
  32B = 256B, single int16 index space).  Horner flushes/AllGathers move
  3.2MB instead of 12.8MB, and the initial x~ table is host-provided so the
  first flush+AllGather disappears.
- gcn_norm folded: dis[src] into table writes, dis[dst] into the edge masks
  (dis computed on host; no on-device degree phase).
"""

import os
import sys
import types

import numpy as np
from ml_dtypes import bfloat16

# ----------------------------------------------------------------------------
# problem constants (hardcoded; kernel.py must be self-contained)
N = 100000
E = 1600000
K = 3
T = 4
H = 16
BN_EPS = 1e-5
NCORE = 8
SHARD = 12500
TPC = 98                 # tiles per core (12544 rows)
ROWS = TPC * 128         # 12544
NT = NCORE * ROWS        # 100352 table rows
NQUAD = NT // 4          # 25088 quad elements (wide)
NOCT = NT // 8           # 12544 oct elements (narrow table)
CPC = 8                  # columns per gather call (8*128 = 1024 slots)
CALL = CPC * 128
IW = CALL // 16          # idx words per call per partition
SEGCAP = 88              # max data columns per segment (pre-padding)

_EXEC_NS = [None]


def _install_hookshim():
    if "antenv.axon_hooks" in sys.modules:
        return
    try:
        import antenv
    except ImportError:
        return
    mod = types.ModuleType("antenv.axon_hooks")
    mod._hook = None
    mod.set_axon_ntff_profile_hook = lambda h: setattr(mod, "_hook", h)
    mod.get_axon_ntff_profile_hook = lambda: mod._hook
    sys.modules["antenv.axon_hooks"] = mod
    antenv.axon_hooks = mod
    try:
        from trn_agent_boot.trn_boot import _ntff_profile_via_ctypes
        hook = _ntff_profile_via_ctypes("/opt/axon/libaxon_pjrt.so")
        if hook is not None:
            mod.set_axon_ntff_profile_hook(hook)
    except Exception:
        pass


# ----------------------------------------------------------------------------
def _build_layout(cnt_all):
    """cnt_all: [8, ROWS] per-dst edge counts in rank order.  Shared layout."""
    D = np.zeros(TPC, np.int64)
    for c in range(NCORE):
        for t in range(TPC):
            D[t] = max(D[t], cnt_all[c][t * 128:(t + 1) * 128].max(initial=0))
    ds = [(t, int(D[t])) for t in range(TPC) if D[t] > 0]
    n = len(ds)
    CAP = SEGCAP
    INF = 1 << 30
    dp = [INF] * (n + 1)
    dp[n] = 0
    nxt = [0] * (n + 1)
    for i in range(n - 1, -1, -1):
        tot = 0
        for j in range(i + 1, min(i + 15, n + 1)):
            tot += ds[j - 1][1]
            if tot > CAP:
                break
            c = (tot + CPC - 1) // CPC * CPC + CPC + dp[j]
            if c < dp[i]:
                dp[i] = c
                nxt[i] = j
    segs = []
    i = 0
    while i < n:
        j = nxt[i]
        cur, off = [], 0
        for k in range(i, j):
            cur.append((ds[k][0], off, ds[k][1]))
            off += ds[k][1]
        segs.append((cur, off))
        i = j
    out = []
    col0 = 0
    call0 = 0
    for tiles, ncols_raw in segs:
        ncols = ncols_raw + (-ncols_raw % CPC)
        out.append(dict(tiles=tiles, ncols=ncols, col0=col0, call0=call0,
                        ncalls=ncols // CPC))
        col0 += ncols
        call0 += ncols // CPC
    return dict(D=D, segs=out, ncols=col0, ncalls=call0)


def _host_prep(x, edge_index, edge_weight):
    """Everything host-side: norm, rank layout, ELL columns, idx streams,
    masks, initial narrow table."""
    src = np.asarray(edge_index[0], np.int64)
    dst = np.asarray(edge_index[1], np.int64)
    ew = np.asarray(edge_weight, np.float64)
    xv = np.asarray(x, np.float64).reshape(-1)

    deg = np.bincount(dst, weights=ew, minlength=N)
    dis = np.where(deg > 0, 1.0 / np.sqrt(np.maximum(deg, 1e-30)), 0.0)

    counts = np.bincount(dst, minlength=N)
    order, rank, G = [], [], np.empty(N, np.int64)
    for c in range(NCORE):
        cc = np.zeros(ROWS, np.int64)
        cc[:SHARD] = counts[c * SHARD:(c + 1) * SHARD]
        o = np.argsort(-cc, kind="stable")      # rank i -> padded-local node
        r = np.empty(ROWS, np.int64)
        r[o] = np.arange(ROWS)
        order.append(o)
        rank.append(r)
        G[c * SHARD:(c + 1) * SHARD] = c * ROWS + r[:SHARD]

    gsrc = G[src]
    owner = dst // SHARD

    cnt_all = np.zeros((NCORE, ROWS), np.int64)
    for c in range(NCORE):
        m = owner == c
        gr = rank[c][dst[m] - c * SHARD]
        cnt_all[c] = np.bincount(gr, minlength=ROWS)
    layout = _build_layout(cnt_all)
    segs, NCOLS, NCALLS = layout["segs"], layout["ncols"], layout["ncalls"]

    colbase = np.zeros(TPC, np.int64)
    for s in segs:
        for (t, off, d) in s["tiles"]:
            colbase[t] = s["col0"] + off

    in_maps = []
    for c in range(NCORE):
        m = owner == c
        ls = dst[m] - c * SHARD
        sg = gsrc[m]
        ws = (ew[m] * dis[dst[m]]).astype(np.float64)   # mask = w * dis[dst]
        gr = rank[c][ls]
        srcs = src[m]
        oe = np.argsort(gr, kind="stable")
        gr, sg, ws, srcs = gr[oe], sg[oe], ws[oe], srcs[oe]
        bc = np.bincount(gr, minlength=ROWS)
        starts = np.concatenate([[0], np.cumsum(bc)[:-1]])
        d_within = np.arange(len(gr)) - np.repeat(starts, bc)
        tl = gr // 128
        p = gr % 128
        col = colbase[tl] + d_within

        qidxcol = np.zeros((NCOLS, 128), np.int16)   # wide quad idx
        nidxcol = np.zeros((NCOLS, 128), np.int16)   # narrow oct idx
        wqq = np.zeros((128, NCOLS, 4), np.float32)
        wqn = np.zeros((128, NCOLS, 8), np.float32)
        xell = np.zeros((128, NCOLS), np.float32)
        wq1 = np.zeros((128, NCOLS), np.float32)
        qidxcol[col, p] = (sg // 4).astype(np.int16)
        nidxcol[col, p] = (sg // 8).astype(np.int16)
        wqq[p, col, sg % 4] = ws
        wqn[p, col, sg % 8] = ws
        xell[p, col] = (xv * dis)[srcs]
        wq1[p, col] = ws

        def wrap(colmat):
            qc = colmat.reshape(NCALLS, CPC * 128)
            w16 = qc.reshape(NCALLS, IW, 16).transpose(0, 2, 1)
            return np.ascontiguousarray(
                np.tile(w16, (1, 8, 1)).transpose(1, 0, 2).reshape(
                    128, NCALLS * IW))

        qidx = wrap(qidxcol)
        nidx = wrap(nidxcol)

        # wide home uploads
        xs = np.zeros((128, TPC), np.float32)
        dsh = np.zeros((128, TPC), np.float32)
        rmw = np.zeros((128, TPC), np.float32)
        loc = np.minimum(order[c], SHARD - 1)
        vals_x = np.where(order[c] < SHARD, xv[c * SHARD + loc], 0.0)
        vals_d = np.where(order[c] < SHARD, dis[c * SHARD + loc], 0.0)
        i = np.arange(ROWS)
        xs[i % 128, i // 128] = vals_x
        dsh[i % 128, i // 128] = vals_d
        rmw[i % 128, i // 128] = (order[c] < SHARD).astype(np.float32)

        in_maps.append({
            "xsh": xs,
            "dish": dsh,
            "rmask": rmw,
            "qidx": qidx,
            "nidx": nidx,
            "wqq": wqq.astype(bfloat16),
            "wqn": wqn.astype(bfloat16),
            "xell": xell,
            "wq1": wq1.astype(bfloat16),
        })

    return in_maps, layout, order


# ----------------------------------------------------------------------------
def _build_bass(layout):
    import concourse.bass as bass
    import concourse.mybir as mybir
    import concourse.tile as tile
    from concourse import bacc

    F32 = mybir.dt.float32
    BF16 = mybir.dt.bfloat16
    I16 = mybir.dt.int16
    AO = mybir.AluOpType
    AF = mybir.ActivationFunctionType
    AX = mybir.AxisListType

    segs, NCOLS, NCALLS = layout["segs"], layout["ncols"], layout["ncalls"]
    RG = [list(range(NCORE))]

    nc = bacc.Bacc("TRN2", target_bir_lowering=False, debug=False,
                   num_devices=NCORE, num_swdge_queues=4)

    xsh_d = nc.dram_tensor("xsh", [128, TPC], F32, kind="ExternalInput").ap()
    dish_d = nc.dram_tensor("dish", [128, TPC], F32, kind="ExternalInput").ap()
    rmask_d = nc.dram_tensor("rmask", [128, TPC], F32, kind="ExternalInput").ap()
    qidx_d = nc.dram_tensor("qidx", [128, NCALLS * IW], I16, kind="ExternalInput").ap()
    nidx_d = nc.dram_tensor("nidx", [128, NCALLS * IW], I16, kind="ExternalInput").ap()
    wqq_d = nc.dram_tensor("wqq", [128, NCOLS, 4], BF16, kind="ExternalInput").ap()
    wqn_d = nc.dram_tensor("wqn", [128, NCOLS, 8], BF16, kind="ExternalInput").ap()
    xell_d = nc.dram_tensor("xell", [128, NCOLS], F32, kind="ExternalInput").ap()
    wq1_d = nc.dram_tensor("wq1", [128, NCOLS], BF16, kind="ExternalInput").ap()
    coef_d = nc.dram_tensor("coef", [128, 320], F32, kind="ExternalInput").ap()
    wrow_d = nc.dram_tensor("wrow", [128, 768], F32, kind="ExternalInput").ap()
    out_d = nc.dram_tensor("out", [128, TPC], F32, kind="ExternalOutput").ap()

    qrr = [0]

    def next_q():
        q = qrr[0] % 4
        qrr[0] += 1
        return q

    with tile.TileContext(nc) as tc:
        with (
            tc.tile_pool(name="pers", bufs=1) as pp,
            tc.tile_pool(name="qseg", bufs=2) as qp,
            tc.tile_pool(name="stage", bufs=2) as sp,
            tc.tile_pool(name="nmask", bufs=2) as nmp,
            tc.tile_pool(name="tmpw", bufs=1) as tp,
            tc.tile_pool(name="tmpe", bufs=1) as tpe,
            tc.tile_pool(name="psum", bufs=1, space="PSUM") as psp,
            tc.tile_pool(name="dram", bufs=1, space="DRAM") as dp,
        ):
            # persistent tiles
            xsh = pp.tile([128, TPC], F32)
            dish = pp.tile([128, TPC], F32)
            rmask = pp.tile([128, TPC], F32)
            wqq = pp.tile([128, NCOLS, 4], BF16)
            coef = pp.tile([128, 320], F32)
            wrowT = pp.tile([128, 768], F32)
            X = pp.tile([128, TPC, 48], BF16)
            P = pp.tile([128, TPC, 48], F32)
            Z = pp.tile([128, TPC, 48], F32)
            hf = pp.tile([128, TPC, 16], F32)
            tw = pp.tile([128, TPC, 64], BF16)
            ntw = pp.tile([128, TPC, 8], F32)
            nar = pp.tile([128, TPC], F32)
            acc = pp.tile([128, TPC], F32)
            vt = pp.tile([128, TPC, 5], F32)
            sums = pp.tile([128, 32], F32)
            bnst = pp.tile([128, 32], F32)
            s16a = pp.tile([128, 16], F32)
            s16b = pp.tile([128, 16], F32)
            s16c = pp.tile([128, 16], F32)

            tabA = dp.tile([NT, 64], BF16)
            tabB = dp.tile([NT, 64], BF16)
            tabAs = nc.dram_tensor("tabAs", [NT, 64], BF16, kind="Internal",
                                   addr_space="Shared")
            tabBs = nc.dram_tensor("tabBs", [NT, 64], BF16, kind="Internal",
                                   addr_space="Shared")
            tin = dp.tile([ROWS, 64], BF16)
            ntAs = nc.dram_tensor("ntAs", [NT, 8], F32, kind="Internal",
                                  addr_space="Shared")
            ntBs = nc.dram_tensor("ntBs", [NT, 8], F32, kind="Internal",
                                  addr_space="Shared")
            ntin = dp.tile([ROWS, 8], F32)
            bnb1 = dp.tile([1, 32], F32)
            bnb2 = dp.tile([1, 32], F32)

            def cap(i):  # coef scalar AP [128,1]
                return coef[:, i:i + 1]

            # loads
            nc.sync.dma_start(xsh[:], xsh_d[:])
            nc.sync.dma_start(dish[:], dish_d[:])
            nc.sync.dma_start(rmask[:], rmask_d[:])
            nc.sync.dma_start(wqq[:], wqq_d[:])
            nc.sync.dma_start(coef[:], coef_d[:])
            nc.sync.dma_start(wrowT[:], wrow_d[:])

            nc.vector.memset(P[:], 0.0)
            nc.vector.memset(nar[:], 0.0)
            nc.vector.memset(tw[:], 0.0)
            nc.vector.memset(ntw[:], 0.0)

            # ---------------- gather sweeps --------------------------------
            def sweep_wide_fused(tab_ap, tabn_ap):
                build_root_into_Z()
                tabq = tab_ap.rearrange("(q f) c -> q (f c)", f=4)
                tinv = tin[:].rearrange("(t p) c -> p t c", p=128)
                for s in segs:
                    ncalls = s["ncalls"]
                    qs = qp.tile([128, ncalls * IW], I16, tag="qs")
                    nc.sync.dma_start(
                        qs[:], qidx_d[:, s["call0"] * IW:(s["call0"] + ncalls) * IW])
                    st = sp.tile([128, s["ncols"], 256], BF16, tag="st")
                    for ci in range(ncalls):
                        nc.gpsimd.dma_gather(
                            st[:, CPC * ci:CPC * (ci + 1), :], tabq,
                            qs[:, ci * IW:(ci + 1) * IW],
                            CALL, CALL, 256, single_packet=True,
                            queue_num=next_q())
                    nc_s = s["ncols"]
                    c0s = s["col0"]
                    sv = st[:, 0:nc_s, :].rearrange(
                        "p d (q j) -> p d q j", q=4)[:, :, :, 0:48]
                    wv = wqq[:, c0s:c0s + nc_s, :].rearrange(
                        "p d (q u) -> p d q u", u=1).to_broadcast(
                        [128, nc_s, 4, 48])
                    nc.vector.tensor_tensor(sv, sv, wv, AO.mult)
                    tmpq = tp.tile([128, 96, 48], BF16, tag="qred")
                    nc.vector.tensor_tensor(
                        tmpq[:, 0:nc_s, :], st[:, 0:nc_s, 0:48],
                        st[:, 0:nc_s, 64:112], AO.add)
                    nc.vector.tensor_tensor(
                        tmpq[:, 0:nc_s, :], tmpq[:, 0:nc_s, :],
                        st[:, 0:nc_s, 128:176], AO.add)
                    nc.vector.tensor_tensor(
                        tmpq[:, 0:nc_s, :], tmpq[:, 0:nc_s, :],
                        st[:, 0:nc_s, 192:240], AO.add)
                    for (t, off, d) in s["tiles"]:
                        nc.vector.tensor_reduce(
                            P[:, t, :],
                            tmpq[:, off:off + d, :].rearrange("p d j -> p j d"),
                            axis=AX.X, op=AO.add)
                    # fused combine + flush for this segment's tile range
                    t0 = s["tiles"][0][0]
                    t1 = s["tiles"][-1][0] + 1
                    ntl = t1 - t0
                    for k in range(K):
                        tmpE = tpe.tile([128, 14, 16, 16], BF16, tag="tmpE")
                        pb = P[:, t0:t1, k * 16:(k + 1) * 16].rearrange(
                            "p t (u i) -> p t u i", u=1).to_broadcast(
                            [128, ntl, 16, 16])
                        wv2 = wrowT[:, k * 256:(k + 1) * 256].rearrange(
                            "p (u o i) -> p u o i", u=1, o=16).to_broadcast(
                            [128, ntl, 16, 16])
                        nc.vector.tensor_tensor(tmpE[:, 0:ntl], pb, wv2, AO.mult)
                        zt = tpe.tile([128, 14, 16], F32, tag="ztE")
                        nc.vector.tensor_reduce(zt[:, 0:ntl], tmpE[:, 0:ntl],
                                                axis=AX.X, op=AO.add)
                        nc.vector.tensor_tensor(
                            Z[:, t0:t1, k * 16:(k + 1) * 16],
                            Z[:, t0:t1, k * 16:(k + 1) * 16],
                            zt[:, 0:ntl], AO.add)
                    nc.scalar.activation(X[:, t0:t1, :], Z[:, t0:t1, :], AF.Relu)
                    if tabn_ap is not None:
                        nc.vector.tensor_tensor(
                            tw[:, t0:t1, 0:48], X[:, t0:t1, :],
                            dish[:, t0:t1].rearrange(
                                "p (t u) -> p t u", u=1).to_broadcast(
                                [128, ntl, 48]), AO.mult)
                        nc.sync.dma_start(tinv[:, t0:t1, :], tw[:, t0:t1, :])
                if tabn_ap is not None:
                    nc.gpsimd.collective_compute(
                        "AllGather", AO.bypass, ins=[tin[:].opt()],
                        outs=[tabn_ap.opt()], replica_groups=RG)

            def sweep_narrow(ntab_ap):
                ntq = ntab_ap.rearrange("(q f) c -> q (f c)", f=8)
                for s in segs:
                    ncalls = s["ncalls"]
                    qs = qp.tile([128, ncalls * IW], I16, tag="nqs")
                    nc.sync.dma_start(
                        qs[:], nidx_d[:, s["call0"] * IW:(s["call0"] + ncalls) * IW])
                    wn = nmp.tile([128, SEGCAP + CPC, 8], BF16, tag="wn")
                    nc.sync.dma_start(
                        wn[:, 0:s["ncols"], :],
                        wqn_d[:, s["col0"]:s["col0"] + s["ncols"], :])
                    stn = sp.tile([128, s["ncols"], 64], F32, tag="st")
                    for ci in range(ncalls):
                        nc.gpsimd.dma_gather(
                            stn[:, CPC * ci:CPC * (ci + 1), :], ntq,
                            qs[:, ci * IW:(ci + 1) * IW],
                            CALL, CALL, 64, single_packet=True,
                            queue_num=next_q())
                    for (t, off, d) in s["tiles"]:
                        tmpn = tp.tile([128, SEGCAP, 8], F32, tag="tmpn")
                        mv = stn[:, off:off + d, :].rearrange(
                            "p d (o j) -> p d o j", o=8)[:, :, :, 0]
                        nc.vector.tensor_tensor(
                            tmpn[:, 0:d], mv, wn[:, off:off + d, :], AO.mult)
                        nc.vector.tensor_reduce(
                            nar[:, t:t + 1], tmpn[:, 0:d],
                            axis=AX.XY, op=AO.add)

            def table_flush(tab_ap):
                nc.sync.dma_start(
                    tin[:].rearrange("(t p) c -> p t c", p=128), tw[:])
                nc.gpsimd.collective_compute(
                    "AllGather", AO.bypass, ins=[tin[:].opt()],
                    outs=[tab_ap.opt()], replica_groups=RG)

            def ntable_flush(src_ap, nt_ap):
                # ntw[:, :, 0] = src * dis ; flush
                nc.vector.tensor_tensor(
                    ntw[:, :, 0:1],
                    src_ap.rearrange("p (t u) -> p t u", u=1),
                    dish[:].rearrange("p (t u) -> p t u", u=1),
                    AO.mult)
                nc.sync.dma_start(
                    ntin[:].rearrange("(t p) c -> p t c", p=128), ntw[:])
                nc.gpsimd.collective_compute(
                    "AllGather", AO.bypass, ins=[ntin[:].opt()],
                    outs=[nt_ap.opt()], replica_groups=RG)

            def build_root_into_Z():
                # Z[:, :, c] = x * rootw_c + b_c
                for c in range(48):
                    bb = coef[:, 96 + c:97 + c].rearrange(
                        "p (t u) -> p t u", u=1).to_broadcast([128, TPC, 1])
                    nc.vector.scalar_tensor_tensor(
                        Z[:, :, c:c + 1],
                        xsh[:].rearrange("p (t u) -> p t u", u=1),
                        cap(48 + c), bb, AO.mult, AO.add)

            TG = 14  # einsum tile-group

            def conv1_combine():
                # Z = root + P@Wblk ; X = relu(Z)   (P already dis[dst]-scaled)
                build_root_into_Z()
                for g0 in range(0, TPC, TG):
                    tg = min(TG, TPC - g0)
                    for k in range(K):
                        tmpE = tpe.tile([128, TG, 16, 16], BF16, tag="tmpE")
                        pb = P[:, g0:g0 + tg, k * 16:(k + 1) * 16].rearrange(
                            "p t (u i) -> p t u i", u=1).to_broadcast(
                            [128, tg, 16, 16])
                        wv = wrowT[:, k * 256:(k + 1) * 256].rearrange(
                            "p (u o i) -> p u o i", u=1, o=16).to_broadcast(
                            [128, tg, 16, 16])
                        nc.vector.tensor_tensor(tmpE[:, 0:tg], pb, wv, AO.mult)
                        zt = tpe.tile([128, TG, 16], F32, tag="ztE")
                        nc.vector.tensor_reduce(zt[:, 0:tg], tmpE[:, 0:tg],
                                                axis=AX.X, op=AO.add)
                        nc.vector.tensor_tensor(
                            Z[:, g0:g0 + tg, k * 16:(k + 1) * 16],
                            Z[:, g0:g0 + tg, k * 16:(k + 1) * 16],
                            zt[:, 0:tg], AO.add)
                nc.scalar.activation(X[:], Z[:], AF.Relu)

            # ---------------- S1: x~ sweep from host-marshalled slots ------
            for s1 in segs:
                nc1 = s1["ncols"]
                xe = qp.tile([128, SEGCAP + CPC], F32, tag="xe")
                nc.sync.dma_start(
                    xe[:, 0:nc1], xell_d[:, s1["col0"]:s1["col0"] + nc1])
                w1t = qp.tile([128, SEGCAP + CPC], BF16, tag="w1")
                nc.sync.dma_start(
                    w1t[:, 0:nc1], wq1_d[:, s1["col0"]:s1["col0"] + nc1])
                tmp1 = tp.tile([128, SEGCAP + CPC], F32, tag="tmp1")
                nc.vector.tensor_tensor(
                    tmp1[:, 0:nc1], xe[:, 0:nc1], w1t[:, 0:nc1], AO.mult)
                for (t, off, d) in s1["tiles"]:
                    nc.vector.tensor_reduce(
                        nar[:, t:t + 1], tmp1[:, off:off + d],
                        axis=AX.X, op=AO.add)
            build_root_into_Z()
            for c in range(48):
                nc.vector.scalar_tensor_tensor(
                    Z[:, :, c:c + 1],
                    nar[:].rearrange("p (t u) -> p t u", u=1),
                    cap(0 + c), Z[:, :, c:c + 1], AO.mult, AO.add)
            nc.scalar.activation(X[:], Z[:], AF.Relu)
            nc.vector.tensor_tensor(
                tw[:, :, 0:48], X[:],
                dish[:].rearrange("p (t u) -> p t u", u=1).to_broadcast(
                    [128, TPC, 48]), AO.mult)
            table_flush(tabAs.ap())

            # ---------------- S2..S4 wide sweeps ---------------------------
            sweep_wide_fused(tabAs.ap(), tabBs.ap())
            sweep_wide_fused(tabBs.ap(), tabAs.ap())
            sweep_wide_fused(tabAs.ap(), None)

            h = X  # reuse X storage for post-BN h (first 16 cols)
            # h1 = mean over stacks
            nc.vector.tensor_tensor(hf[:], X[:, :, 0:16], X[:, :, 16:32], AO.add)
            nc.vector.tensor_tensor(hf[:], hf[:], X[:, :, 32:48], AO.add)
            nc.scalar.activation(hf[:], hf[:], AF.Copy, scale=1.0 / 3.0)

            # ---------------- BatchNorm -----------------------------------
            nc.vector.tensor_tensor(
                hf[:], hf[:],
                rmask[:].rearrange("p (t u) -> p t u", u=1).to_broadcast(
                    [128, TPC, 16]), AO.mult)
            nc.vector.tensor_reduce(
                sums[:, 0:16], hf[:].rearrange("p t f -> p f t"),
                axis=AX.X, op=AO.add)
            nc.scalar.activation(Z[:, :, 0:16], hf[:], AF.Square)
            nc.vector.tensor_reduce(
                sums[:, 16:32], Z[:, :, 0:16].rearrange("p t f -> p f t"),
                axis=AX.X, op=AO.add)
            ones_ps = psp.tile([1, 32], F32)
            nc.tensor.matmul(ones_ps[:], coef[:, 263:264], sums[:],
                             start=True, stop=True)
            bnl = pp.tile([1, 32], F32)
            nc.scalar.activation(bnl[:], ones_ps[:], AF.Copy)
            nc.sync.dma_start(bnb1[:], bnl[:])
            nc.gpsimd.collective_compute(
                "AllReduce", AO.add, ins=[bnb1[:].opt()], outs=[bnb2[:].opt()],
                replica_groups=RG)
            nc.sync.dma_start(bnst[:], bnb2[:].to_broadcast([128, 32]))
            nc.scalar.activation(s16a[:], bnst[:, 0:16], AF.Copy, scale=1.0 / N)
            nc.scalar.activation(s16b[:], bnst[:, 16:32], AF.Copy, scale=1.0 / N)
            musq = pp.tile([128, 16], F32)
            nc.scalar.activation(musq[:], s16a[:], AF.Square)
            nc.vector.tensor_tensor(s16b[:], s16b[:], musq[:], AO.subtract)
            sd = pp.tile([128, 16], F32)
            nc.vector.scalar_tensor_tensor(sd[:], s16b[:], BN_EPS, s16b[:],
                                           AO.add, AO.max)
            nc.scalar.activation(sd[:], sd[:], AF.Sqrt)
            rsd = pp.tile([128, 16], F32)
            nc.vector.reciprocal(rsd[:], sd[:])
            nc.vector.tensor_tensor(s16c[:], rsd[:], coef[:, 144:160], AO.mult)
            shf = pp.tile([128, 16], F32)
            nc.vector.tensor_tensor(shf[:], s16a[:], s16c[:], AO.mult)
            nc.vector.tensor_tensor(shf[:], coef[:, 160:176], shf[:], AO.subtract)
            nc.vector.tensor_tensor(
                hf[:], hf[:],
                s16c[:].rearrange("p (u f) -> p u f", u=1).to_broadcast(
                    [128, TPC, 16]), AO.mult)
            nc.vector.tensor_tensor(
                hf[:], hf[:],
                shf[:].rearrange("p (u f) -> p u f", u=1).to_broadcast(
                    [128, TPC, 16]), AO.add)
            nc.scalar.activation(h[:, :, 0:16], hf[:], AF.Relu)

            # ---------------- conv2 projections ----------------------------
            for j in range(5):
                bb = coef[:, 176 + j:177 + j].rearrange(
                    "p (t u) -> p t u", u=1).to_broadcast([128, TPC, 1])
                nc.vector.scalar_tensor_tensor(
                    vt[:, :, j:j + 1], h[:, :, 0:1],
                    cap(181 + j * 16 + 0), bb, AO.mult, AO.add)
                for i in range(1, 16):
                    nc.vector.scalar_tensor_tensor(
                        vt[:, :, j:j + 1], h[:, :, i:i + 1],
                        cap(181 + j * 16 + i), vt[:, :, j:j + 1],
                        AO.mult, AO.add)

            # ---------------- Horner chain ----------------------------------
            nc.scalar.activation(
                acc[:], vt[:, :, 4:5].rearrange("p t u -> p (t u)"), AF.Copy)
            nts = [ntAs, ntBs]
            for step, j in enumerate((3, 2, 1, 0)):
                nt = nts[step % 2]
                ntable_flush(acc[:], nt.ap())
                sweep_narrow(nt.ap())
                nc.vector.tensor_tensor(
                    acc[:], nar[:],
                    vt[:, :, j:j + 1].rearrange("p t u -> p (t u)"), AO.add)

            # ---------------- final linear + sigmoid ------------------------
            outb = pp.tile([128, TPC], F32)
            nc.scalar.activation(outb[:], acc[:], AF.Sigmoid,
                                 scale=cap(261), bias=cap(262))
            nc.sync.dma_start(out_d[:], outb[:])

    nc.compile()
    return nc


# ----------------------------------------------------------------------------
def kernel(x, edge_index, edge_weight, w1_init, w1_w, w1_root, w1_b,
           bn1_g, bn1_b, w2_init, w2_w, w2_root, w2_b, lin_w, lin_b):
    _install_hookshim()
    x = np.asarray(x, np.float32)
    edge_index = np.asarray(edge_index)
    edge_weight = np.asarray(edge_weight, np.float32)

    in_maps, layout, order = _host_prep(x, edge_index, edge_weight)

    # ---- coefficient packing (host): tiny-weight derived scalars
    w1_init = np.asarray(w1_init, np.float64)
    w1_w_ = np.asarray(w1_w, np.float64)
    w1_root = np.asarray(w1_root, np.float64)
    w1_b_ = np.asarray(w1_b, np.float64)
    w2_init = np.asarray(w2_init, np.float64)
    w2_w_ = np.asarray(w2_w, np.float64)
    w2_root = np.asarray(w2_root, np.float64)
    w2_b_ = np.asarray(w2_b, np.float64)

    coef = np.zeros(320, np.float64)
    coef[0:48] = w1_init[:, 0, :].reshape(-1)
    coef[48:96] = w1_root[:, 0, :].reshape(-1)
    coef[96:144] = w1_b_.reshape(-1)
    coef[144:160] = np.asarray(bn1_g, np.float64)
    coef[160:176] = np.asarray(bn1_b, np.float64)
    wk = w2_w_[:, 0, 0]
    gmat = np.zeros((5, 16), np.float64)
    beta = np.zeros(5, np.float64)
    gmat[4] = (wk ** 3 / 3.0) @ w2_init[:, :, 0]
    gmat[3] = (wk ** 3 / 3.0) @ w2_root[:, :, 0]; beta[3] = (wk ** 3 / 3.0) @ w2_b_[:, 0]
    gmat[2] = (wk ** 2 / 3.0) @ w2_root[:, :, 0]; beta[2] = (wk ** 2 / 3.0) @ w2_b_[:, 0]
    gmat[1] = (wk / 3.0) @ w2_root[:, :, 0];      beta[1] = (wk / 3.0) @ w2_b_[:, 0]
    gmat[0] = np.ones(3) / 3.0 @ w2_root[:, :, 0]; beta[0] = np.ones(3) / 3.0 @ w2_b_[:, 0]
    coef[176:181] = beta
    coef[181:261] = gmat.reshape(-1)
    coef[261] = np.asarray(lin_w, np.float64)[0, 0]
    coef[262] = np.asarray(lin_b, np.float64)[0]
    coef[263] = 1.0
    coef_np = np.tile(coef.astype(np.float32)[None, :], (128, 1))

    wrow = np.zeros(768, np.float64)
    for k in range(K):
        for o in range(16):
            wrow[(k * 16 + o) * 16:(k * 16 + o) * 16 + 16] = w1_w_[k, :, o]
    wrow_np = np.tile(wrow.astype(np.float32)[None, :], (128, 1))

    for m in in_maps:
        m["coef"] = coef_np
        m["wrow"] = wrow_np

    nc = _build_bass(layout)
    from concourse.bass_utils import run_bass_kernel_spmd
    trace = os.environ.get("BASS_GNN_TRACE", "0") == "1"
    res = run_bass_kernel_spmd(nc, in_maps, core_ids=list(range(NCORE)),
                               trace=trace)
    _EXEC_NS[0] = res.exec_time_ns

    out = np.empty((N, 1), np.float32)
    for c in range(NCORE):
        ob = res.results[c]["out"]        # [128, TPC]
        i = np.arange(ROWS)
        vals = ob[i % 128, i // 128]       # value at rank i
        keep = order[c] < SHARD
        out[c * SHARD + order[c][keep], 0] = vals[keep]
    return out


def last_exec_ns():
    return _EXEC_NS[0]


# revision 18
# speedup vs baseline: 1.3412x; 1.0926x over previous
"""Trainium2 Bass kernel for nn_ArmaNet02 (ARMA GNN, N=100K, E=1.6M, K=3, T=4, H=16).

Strategy (8 NeuronCores, SPMD), v2:
- dst-sharded ELL, wide gathers use bf16 QUADS (4 nodes x 64 cols = 512B,
  int16 quad indices) from the replicated feature table [NT, 64].
- Narrow state (conv1 t=0 input and the conv2 Horner chain) lives in a
  dedicated compact table [NT, 8] fp32; gather elements are OCTs (8 nodes x
  32B = 256B, single int16 index space).  Horner flushes/AllGathers move
  3.2MB instead of 12.8MB, and the initial x~ table is host-provided so the
  first flush+AllGather disappears.
- gcn_norm folded: dis[src] into table writes, dis[dst] into the edge masks
  (dis computed on host; no on-device degree phase).
"""

import os
import sys
import types

import numpy as np
from ml_dtypes import bfloat16

# ----------------------------------------------------------------------------
# problem constants (hardcoded; kernel.py must be self-contained)
N = 100000
E = 1600000
K = 3
T = 4
H = 16
BN_EPS = 1e-5
NCORE = 8
SHARD = 12500
TPC = 98                 # tiles per core (12544 rows)
ROWS = TPC * 128         # 12544
NT = NCORE * ROWS        # 100352 table rows
NQUAD = NT // 4          # 25088 quad elements (wide)
NOCT = NT // 8           # 12544 oct elements (narrow table)
CPC = 8                  # columns per gather call (8*128 = 1024 slots)
CALL = CPC * 128
IW = CALL // 16          # idx words per call per partition
SEGCAP = 88              # max data columns per segment (pre-padding)

_EXEC_NS = [None]


def _install_hookshim():
    if "antenv.axon_hooks" in sys.modules:
        return
    try:
        import antenv
    except ImportError:
        return
    mod = types.ModuleType("antenv.axon_hooks")
    mod._hook = None
    mod.set_axon_ntff_profile_hook = lambda h: setattr(mod, "_hook", h)
    mod.get_axon_ntff_profile_hook = lambda: mod._hook
    sys.modules["antenv.axon_hooks"] = mod
    antenv.axon_hooks = mod
    try:
        from trn_agent_boot.trn_boot import _ntff_profile_via_ctypes
        hook = _ntff_profile_via_ctypes("/opt/axon/libaxon_pjrt.so")
        if hook is not None:
            mod.set_axon_ntff_profile_hook(hook)
    except Exception:
        pass


# ----------------------------------------------------------------------------
def _build_layout(cnt_all):
    """cnt_all: [8, ROWS] per-dst edge counts in rank order.  Shared layout."""
    D = np.zeros(TPC, np.int64)
    for c in range(NCORE):
        for t in range(TPC):
            D[t] = max(D[t], cnt_all[c][t * 128:(t + 1) * 128].max(initial=0))
    ds = [(t, int(D[t])) for t in range(TPC) if D[t] > 0]
    n = len(ds)
    CAP = SEGCAP
    INF = 1 << 30
    dp = [INF] * (n + 1)
    dp[n] = 0
    nxt = [0] * (n + 1)
    for i in range(n - 1, -1, -1):
        tot = 0
        for j in range(i + 1, min(i + 15, n + 1)):
            tot += ds[j - 1][1]
            if tot > CAP:
                break
            c = (tot + CPC - 1) // CPC * CPC + CPC + dp[j]
            if c < dp[i]:
                dp[i] = c
                nxt[i] = j
    segs = []
    i = 0
    while i < n:
        j = nxt[i]
        cur, off = [], 0
        for k in range(i, j):
            cur.append((ds[k][0], off, ds[k][1]))
            off += ds[k][1]
        segs.append((cur, off))
        i = j
    out = []
    col0 = 0
    call0 = 0
    for tiles, ncols_raw in segs:
        ncols = ncols_raw + (-ncols_raw % CPC)
        out.append(dict(tiles=tiles, ncols=ncols, rawcols=ncols_raw,
                        col0=col0, call0=call0, ncalls=ncols // CPC))
        col0 += ncols
        call0 += ncols // CPC
    return dict(D=D, segs=out, ncols=col0, ncalls=call0)


def _host_prep(x, edge_index, edge_weight):
    """Everything host-side: norm, rank layout, ELL columns, idx streams,
    masks, initial narrow table."""
    src = np.asarray(edge_index[0], np.int64)
    dst = np.asarray(edge_index[1], np.int64)
    ew = np.asarray(edge_weight, np.float64)
    xv = np.asarray(x, np.float64).reshape(-1)

    deg = np.bincount(dst, weights=ew, minlength=N)
    dis = np.where(deg > 0, 1.0 / np.sqrt(np.maximum(deg, 1e-30)), 0.0)

    counts = np.bincount(dst, minlength=N)
    order, rank, G = [], [], np.empty(N, np.int64)
    for c in range(NCORE):
        cc = np.zeros(ROWS, np.int64)
        cc[:SHARD] = counts[c * SHARD:(c + 1) * SHARD]
        o = np.argsort(-cc, kind="stable")      # rank i -> padded-local node
        r = np.empty(ROWS, np.int64)
        r[o] = np.arange(ROWS)
        order.append(o)
        rank.append(r)
        G[c * SHARD:(c + 1) * SHARD] = c * ROWS + r[:SHARD]

    gsrc = G[src]
    owner = dst // SHARD

    cnt_all = np.zeros((NCORE, ROWS), np.int64)
    for c in range(NCORE):
        m = owner == c
        gr = rank[c][dst[m] - c * SHARD]
        cnt_all[c] = np.bincount(gr, minlength=ROWS)
    layout = _build_layout(cnt_all)
    segs, NCOLS, NCALLS = layout["segs"], layout["ncols"], layout["ncalls"]

    colbase = np.zeros(TPC, np.int64)
    for s in segs:
        for (t, off, d) in s["tiles"]:
            colbase[t] = s["col0"] + off

    in_maps = []
    for c in range(NCORE):
        m = owner == c
        ls = dst[m] - c * SHARD
        sg = gsrc[m]
        ws = (ew[m] * dis[dst[m]]).astype(np.float64)   # mask = w * dis[dst]
        gr = rank[c][ls]
        srcs = src[m]
        oe = np.argsort(gr, kind="stable")
        gr, sg, ws, srcs = gr[oe], sg[oe], ws[oe], srcs[oe]
        bc = np.bincount(gr, minlength=ROWS)
        starts = np.concatenate([[0], np.cumsum(bc)[:-1]])
        d_within = np.arange(len(gr)) - np.repeat(starts, bc)
        tl = gr // 128
        p = gr % 128
        col = colbase[tl] + d_within

        qidxcol = np.zeros((NCOLS, 128), np.int16)   # wide quad idx
        nidxcol = np.zeros((NCOLS, 128), np.int16)   # narrow oct idx
        wqq = np.zeros((128, NCOLS, 4), np.float32)
        wqn = np.zeros((128, NCOLS, 8), np.float32)
        xell = np.zeros((128, NCOLS), np.float32)
        wq1 = np.zeros((128, NCOLS), np.float32)
        qidxcol[col, p] = (sg // 4).astype(np.int16)
        nidxcol[col, p] = (sg // 8).astype(np.int16)
        wqq[p, col, sg % 4] = ws
        wqn[p, col, sg % 8] = ws
        xell[p, col] = (xv * dis)[srcs]
        wq1[p, col] = ws

        def wrap(colmat):
            qc = colmat.reshape(NCALLS, CPC * 128)
            w16 = qc.reshape(NCALLS, IW, 16).transpose(0, 2, 1)
            return np.ascontiguousarray(
                np.tile(w16, (1, 8, 1)).transpose(1, 0, 2).reshape(
                    128, NCALLS * IW))

        qidx = wrap(qidxcol)
        nidx = wrap(nidxcol)

        # wide home uploads
        xs = np.zeros((128, TPC), np.float32)
        dsh = np.zeros((128, TPC), np.float32)
        rmw = np.zeros((128, TPC), np.float32)
        loc = np.minimum(order[c], SHARD - 1)
        vals_x = np.where(order[c] < SHARD, xv[c * SHARD + loc], 0.0)
        vals_d = np.where(order[c] < SHARD, dis[c * SHARD + loc], 0.0)
        i = np.arange(ROWS)
        xs[i % 128, i // 128] = vals_x
        dsh[i % 128, i // 128] = vals_d
        rmw[i % 128, i // 128] = (order[c] < SHARD).astype(np.float32)

        in_maps.append({
            "xsh": xs,
            "dish": dsh,
            "rmask": rmw,
            "qidx": qidx,
            "nidx": nidx,
            "wqq": wqq.astype(bfloat16),
            "wqn": wqn.astype(bfloat16),
            "xell": xell,
            "wq1": wq1.astype(bfloat16),
        })

    return in_maps, layout, order


# ----------------------------------------------------------------------------
def _build_bass(layout):
    import concourse.bass as bass
    import concourse.mybir as mybir
    import concourse.tile as tile
    from concourse import bacc

    F32 = mybir.dt.float32
    BF16 = mybir.dt.bfloat16
    I16 = mybir.dt.int16
    AO = mybir.AluOpType
    AF = mybir.ActivationFunctionType
    AX = mybir.AxisListType

    segs, NCOLS, NCALLS = layout["segs"], layout["ncols"], layout["ncalls"]
    RG = [list(range(NCORE))]

    nc = bacc.Bacc("TRN2", target_bir_lowering=False, debug=False,
                   num_devices=NCORE, num_swdge_queues=4)

    xsh_d = nc.dram_tensor("xsh", [128, TPC], F32, kind="ExternalInput").ap()
    dish_d = nc.dram_tensor("dish", [128, TPC], F32, kind="ExternalInput").ap()
    rmask_d = nc.dram_tensor("rmask", [128, TPC], F32, kind="ExternalInput").ap()
    qidx_d = nc.dram_tensor("qidx", [128, NCALLS * IW], I16, kind="ExternalInput").ap()
    nidx_d = nc.dram_tensor("nidx", [128, NCALLS * IW], I16, kind="ExternalInput").ap()
    wqq_d = nc.dram_tensor("wqq", [128, NCOLS, 4], BF16, kind="ExternalInput").ap()
    wqn_d = nc.dram_tensor("wqn", [128, NCOLS, 8], BF16, kind="ExternalInput").ap()
    xell_d = nc.dram_tensor("xell", [128, NCOLS], F32, kind="ExternalInput").ap()
    wq1_d = nc.dram_tensor("wq1", [128, NCOLS], BF16, kind="ExternalInput").ap()
    coef_d = nc.dram_tensor("coef", [128, 320], F32, kind="ExternalInput").ap()
    wrow_d = nc.dram_tensor("wrow", [128, 768], F32, kind="ExternalInput").ap()
    out_d = nc.dram_tensor("out", [128, TPC], F32, kind="ExternalOutput").ap()

    qrr = [0]

    def next_q():
        q = qrr[0] % 4
        qrr[0] += 1
        return q

    with tile.TileContext(nc) as tc:
        with (
            tc.tile_pool(name="pers", bufs=1) as pp,
            tc.tile_pool(name="qseg", bufs=2) as qp,
            tc.tile_pool(name="stage", bufs=2) as sp,
            tc.tile_pool(name="nmask", bufs=2) as nmp,
            tc.tile_pool(name="tmpw", bufs=1) as tp,
            tc.tile_pool(name="tmpe", bufs=1) as tpe,
            tc.tile_pool(name="psum", bufs=1, space="PSUM") as psp,
            tc.tile_pool(name="dram", bufs=1, space="DRAM") as dp,
        ):
            # persistent tiles
            xsh = pp.tile([128, TPC], F32)
            dish = pp.tile([128, TPC], F32)
            rmask = pp.tile([128, TPC], F32)
            wqq = pp.tile([128, NCOLS, 4], BF16)
            coef = pp.tile([128, 320], F32)
            wrowT = pp.tile([128, 768], F32)
            X = pp.tile([128, TPC, 48], BF16)
            P = pp.tile([128, TPC, 48], F32)
            Z = pp.tile([128, TPC, 48], F32)
            hf = pp.tile([128, TPC, 16], F32)
            tw = pp.tile([128, TPC, 64], BF16)
            ntw = pp.tile([128, TPC, 8], F32)
            nar = pp.tile([128, TPC], F32)
            acc = pp.tile([128, TPC], F32)
            vt = pp.tile([128, TPC, 5], F32)
            sums = pp.tile([128, 32], F32)
            bnst = pp.tile([128, 32], F32)
            s16a = pp.tile([128, 16], F32)
            s16b = pp.tile([128, 16], F32)
            s16c = pp.tile([128, 16], F32)

            tabA = dp.tile([NT, 64], BF16)
            tabB = dp.tile([NT, 64], BF16)
            tabAs = nc.dram_tensor("tabAs", [NT, 64], BF16, kind="Internal",
                                   addr_space="Shared")
            tabBs = nc.dram_tensor("tabBs", [NT, 64], BF16, kind="Internal",
                                   addr_space="Shared")
            tin = dp.tile([ROWS, 64], BF16)
            ntAs = nc.dram_tensor("ntAs", [NT, 8], F32, kind="Internal",
                                  addr_space="Shared")
            ntBs = nc.dram_tensor("ntBs", [NT, 8], F32, kind="Internal",
                                  addr_space="Shared")
            ntin = dp.tile([ROWS, 8], F32)
            bnb1 = dp.tile([1, 32], F32)
            bnb2 = dp.tile([1, 32], F32)

            def cap(i):  # coef scalar AP [128,1]
                return coef[:, i:i + 1]

            # loads
            nc.sync.dma_start(xsh[:], xsh_d[:])
            nc.sync.dma_start(dish[:], dish_d[:])
            nc.sync.dma_start(rmask[:], rmask_d[:])
            nc.sync.dma_start(wqq[:], wqq_d[:])
            nc.sync.dma_start(coef[:], coef_d[:])
            nc.sync.dma_start(wrowT[:], wrow_d[:])

            nc.vector.memset(P[:], 0.0)
            nc.vector.memset(nar[:], 0.0)
            nc.vector.memset(tw[:], 0.0)
            nc.vector.memset(ntw[:], 0.0)

            # ---------------- gather sweeps --------------------------------
            def sweep_wide_fused(tab_ap, tabn_ap):
                build_root_into_Z()
                tabq = tab_ap.rearrange("(q f) c -> q (f c)", f=4)
                tinv = tin[:].rearrange("(t p) c -> p t c", p=128)
                for s in segs:
                    ncalls = s["ncalls"]
                    qs = qp.tile([128, ncalls * IW], I16, tag="qs")
                    nc.sync.dma_start(
                        qs[:], qidx_d[:, s["call0"] * IW:(s["call0"] + ncalls) * IW])
                    st = sp.tile([128, s["ncols"], 256], BF16, tag="st")
                    pad = s["ncols"] - s["rawcols"]
                    for ci in range(ncalls):
                        cols = CPC if ci < ncalls - 1 else CPC - pad
                        ni = 128 * cols
                        nc.gpsimd.dma_gather(
                            st[:, CPC * ci:CPC * ci + cols, :], tabq,
                            qs[:, ci * IW:ci * IW + ni // 16],
                            ni, ni, 256, single_packet=True,
                            queue_num=next_q())
                    nc_s = s["ncols"]
                    c0s = s["col0"]
                    sv = st[:, 0:nc_s, :].rearrange(
                        "p d (q j) -> p d q j", q=4)[:, :, :, 0:48]
                    wv = wqq[:, c0s:c0s + nc_s, :].rearrange(
                        "p d (q u) -> p d q u", u=1).to_broadcast(
                        [128, nc_s, 4, 48])
                    nc.vector.tensor_tensor(sv, sv, wv, AO.mult)
                    tmpq = tp.tile([128, 96, 48], BF16, tag="qred")
                    nc.vector.tensor_tensor(
                        tmpq[:, 0:nc_s, :], st[:, 0:nc_s, 0:48],
                        st[:, 0:nc_s, 64:112], AO.add)
                    nc.vector.tensor_tensor(
                        tmpq[:, 0:nc_s, :], tmpq[:, 0:nc_s, :],
                        st[:, 0:nc_s, 128:176], AO.add)
                    nc.vector.tensor_tensor(
                        tmpq[:, 0:nc_s, :], tmpq[:, 0:nc_s, :],
                        st[:, 0:nc_s, 192:240], AO.add)
                    for (t, off, d) in s["tiles"]:
                        nc.vector.tensor_reduce(
                            P[:, t, :],
                            tmpq[:, off:off + d, :].rearrange("p d j -> p j d"),
                            axis=AX.X, op=AO.add)
                    # fused combine + flush for this segment's tile range
                    t0 = s["tiles"][0][0]
                    t1 = s["tiles"][-1][0] + 1
                    ntl = t1 - t0
                    for k in range(K):
                        tmpE = tpe.tile([128, 14, 16, 16], BF16, tag="tmpE")
                        pb = P[:, t0:t1, k * 16:(k + 1) * 16].rearrange(
                            "p t (u i) -> p t u i", u=1).to_broadcast(
                            [128, ntl, 16, 16])
                        wv2 = wrowT[:, k * 256:(k + 1) * 256].rearrange(
                            "p (u o i) -> p u o i", u=1, o=16).to_broadcast(
                            [128, ntl, 16, 16])
                        nc.vector.tensor_tensor(tmpE[:, 0:ntl], pb, wv2, AO.mult)
                        zt = tpe.tile([128, 14, 16], F32, tag="ztE")
                        nc.vector.tensor_reduce(zt[:, 0:ntl], tmpE[:, 0:ntl],
                                                axis=AX.X, op=AO.add)
                        nc.vector.tensor_tensor(
                            Z[:, t0:t1, k * 16:(k + 1) * 16],
                            Z[:, t0:t1, k * 16:(k + 1) * 16],
                            zt[:, 0:ntl], AO.add)
                    nc.scalar.activation(X[:, t0:t1, :], Z[:, t0:t1, :], AF.Relu)
                    if tabn_ap is not None:
                        nc.vector.tensor_tensor(
                            tw[:, t0:t1, 0:48], X[:, t0:t1, :],
                            dish[:, t0:t1].rearrange(
                                "p (t u) -> p t u", u=1).to_broadcast(
                                [128, ntl, 48]), AO.mult)
                        nc.sync.dma_start(tinv[:, t0:t1, :], tw[:, t0:t1, :])
                if tabn_ap is not None:
                    nc.gpsimd.collective_compute(
                        "AllGather", AO.bypass, ins=[tin[:].opt()],
                        outs=[tabn_ap.opt()], replica_groups=RG)

            def sweep_narrow(ntab_ap):
                ntq = ntab_ap.rearrange("(q f) c -> q (f c)", f=8)
                for s in segs:
                    ncalls = s["ncalls"]
                    qs = qp.tile([128, ncalls * IW], I16, tag="nqs")
                    nc.sync.dma_start(
                        qs[:], nidx_d[:, s["call0"] * IW:(s["call0"] + ncalls) * IW])
                    wn = nmp.tile([128, SEGCAP + CPC, 8], BF16, tag="wn")
                    nc.sync.dma_start(
                        wn[:, 0:s["ncols"], :],
                        wqn_d[:, s["col0"]:s["col0"] + s["ncols"], :])
                    stn = sp.tile([128, s["ncols"], 64], F32, tag="st")
                    pad = s["ncols"] - s["rawcols"]
                    for ci in range(ncalls):
                        cols = CPC if ci < ncalls - 1 else CPC - pad
                        ni = 128 * cols
                        nc.gpsimd.dma_gather(
                            stn[:, CPC * ci:CPC * ci + cols, :], ntq,
                            qs[:, ci * IW:ci * IW + ni // 16],
                            ni, ni, 64, single_packet=True,
                            queue_num=next_q())
                    for (t, off, d) in s["tiles"]:
                        tmpn = tp.tile([128, SEGCAP, 8], F32, tag="tmpn")
                        mv = stn[:, off:off + d, :].rearrange(
                            "p d (o j) -> p d o j", o=8)[:, :, :, 0]
                        nc.vector.tensor_tensor(
                            tmpn[:, 0:d], mv, wn[:, off:off + d, :], AO.mult)
                        nc.vector.tensor_reduce(
                            nar[:, t:t + 1], tmpn[:, 0:d],
                            axis=AX.XY, op=AO.add)

            def table_flush(tab_ap):
                nc.sync.dma_start(
                    tin[:].rearrange("(t p) c -> p t c", p=128), tw[:])
                nc.gpsimd.collective_compute(
                    "AllGather", AO.bypass, ins=[tin[:].opt()],
                    outs=[tab_ap.opt()], replica_groups=RG)

            def ntable_flush(src_ap, nt_ap):
                # ntw[:, :, 0] = src * dis ; flush
                nc.vector.tensor_tensor(
                    ntw[:, :, 0:1],
                    src_ap.rearrange("p (t u) -> p t u", u=1),
                    dish[:].rearrange("p (t u) -> p t u", u=1),
                    AO.mult)
                nc.sync.dma_start(
                    ntin[:].rearrange("(t p) c -> p t c", p=128), ntw[:])
                nc.gpsimd.collective_compute(
                    "AllGather", AO.bypass, ins=[ntin[:].opt()],
                    outs=[nt_ap.opt()], replica_groups=RG)

            def build_root_into_Z():
                # Z[:, :, c] = x * rootw_c + b_c
                for c in range(48):
                    bb = coef[:, 96 + c:97 + c].rearrange(
                        "p (t u) -> p t u", u=1).to_broadcast([128, TPC, 1])
                    nc.vector.scalar_tensor_tensor(
                        Z[:, :, c:c + 1],
                        xsh[:].rearrange("p (t u) -> p t u", u=1),
                        cap(48 + c), bb, AO.mult, AO.add)

            TG = 14  # einsum tile-group

            def conv1_combine():
                # Z = root + P@Wblk ; X = relu(Z)   (P already dis[dst]-scaled)
                build_root_into_Z()
                for g0 in range(0, TPC, TG):
                    tg = min(TG, TPC - g0)
                    for k in range(K):
                        tmpE = tpe.tile([128, TG, 16, 16], BF16, tag="tmpE")
                        pb = P[:, g0:g0 + tg, k * 16:(k + 1) * 16].rearrange(
                            "p t (u i) -> p t u i", u=1).to_broadcast(
                            [128, tg, 16, 16])
                        wv = wrowT[:, k * 256:(k + 1) * 256].rearrange(
                            "p (u o i) -> p u o i", u=1, o=16).to_broadcast(
                            [128, tg, 16, 16])
                        nc.vector.tensor_tensor(tmpE[:, 0:tg], pb, wv, AO.mult)
                        zt = tpe.tile([128, TG, 16], F32, tag="ztE")
                        nc.vector.tensor_reduce(zt[:, 0:tg], tmpE[:, 0:tg],
                                                axis=AX.X, op=AO.add)
                        nc.vector.tensor_tensor(
                            Z[:, g0:g0 + tg, k * 16:(k + 1) * 16],
                            Z[:, g0:g0 + tg, k * 16:(k + 1) * 16],
                            zt[:, 0:tg], AO.add)
                nc.scalar.activation(X[:], Z[:], AF.Relu)

            # ---------------- S1: x~ sweep from host-marshalled slots ------
            for s1 in segs:
                nc1 = s1["ncols"]
                xe = qp.tile([128, SEGCAP + CPC], F32, tag="xe")
                nc.sync.dma_start(
                    xe[:, 0:nc1], xell_d[:, s1["col0"]:s1["col0"] + nc1])
                w1t = qp.tile([128, SEGCAP + CPC], BF16, tag="w1")
                nc.sync.dma_start(
                    w1t[:, 0:nc1], wq1_d[:, s1["col0"]:s1["col0"] + nc1])
                tmp1 = tp.tile([128, SEGCAP + CPC], F32, tag="tmp1")
                nc.vector.tensor_tensor(
                    tmp1[:, 0:nc1], xe[:, 0:nc1], w1t[:, 0:nc1], AO.mult)
                for (t, off, d) in s1["tiles"]:
                    nc.vector.tensor_reduce(
                        nar[:, t:t + 1], tmp1[:, off:off + d],
                        axis=AX.X, op=AO.add)
            build_root_into_Z()
            for c in range(48):
                nc.vector.scalar_tensor_tensor(
                    Z[:, :, c:c + 1],
                    nar[:].rearrange("p (t u) -> p t u", u=1),
                    cap(0 + c), Z[:, :, c:c + 1], AO.mult, AO.add)
            nc.scalar.activation(X[:], Z[:], AF.Relu)
            nc.vector.tensor_tensor(
                tw[:, :, 0:48], X[:],
                dish[:].rearrange("p (t u) -> p t u", u=1).to_broadcast(
                    [128, TPC, 48]), AO.mult)
            table_flush(tabAs.ap())

            # ---------------- S2..S4 wide sweeps ---------------------------
            sweep_wide_fused(tabAs.ap(), tabBs.ap())
            sweep_wide_fused(tabBs.ap(), tabAs.ap())
            sweep_wide_fused(tabAs.ap(), None)

            h = X  # reuse X storage for post-BN h (first 16 cols)
            # h1 = mean over stacks
            nc.vector.tensor_tensor(hf[:], X[:, :, 0:16], X[:, :, 16:32], AO.add)
            nc.vector.tensor_tensor(hf[:], hf[:], X[:, :, 32:48], AO.add)
            nc.scalar.activation(hf[:], hf[:], AF.Copy, scale=1.0 / 3.0)

            # ---------------- BatchNorm -----------------------------------
            nc.vector.tensor_tensor(
                hf[:], hf[:],
                rmask[:].rearrange("p (t u) -> p t u", u=1).to_broadcast(
                    [128, TPC, 16]), AO.mult)
            nc.vector.tensor_reduce(
                sums[:, 0:16], hf[:].rearrange("p t f -> p f t"),
                axis=AX.X, op=AO.add)
            nc.scalar.activation(Z[:, :, 0:16], hf[:], AF.Square)
            nc.vector.tensor_reduce(
                sums[:, 16:32], Z[:, :, 0:16].rearrange("p t f -> p f t"),
                axis=AX.X, op=AO.add)
            ones_ps = psp.tile([1, 32], F32)
            nc.tensor.matmul(ones_ps[:], coef[:, 263:264], sums[:],
                             start=True, stop=True)
            bnl = pp.tile([1, 32], F32)
            nc.scalar.activation(bnl[:], ones_ps[:], AF.Copy)
            nc.sync.dma_start(bnb1[:], bnl[:])
            nc.gpsimd.collective_compute(
                "AllReduce", AO.add, ins=[bnb1[:].opt()], outs=[bnb2[:].opt()],
                replica_groups=RG)
            nc.sync.dma_start(bnst[:], bnb2[:].to_broadcast([128, 32]))
            nc.scalar.activation(s16a[:], bnst[:, 0:16], AF.Copy, scale=1.0 / N)
            nc.scalar.activation(s16b[:], bnst[:, 16:32], AF.Copy, scale=1.0 / N)
            musq = pp.tile([128, 16], F32)
            nc.scalar.activation(musq[:], s16a[:], AF.Square)
            nc.vector.tensor_tensor(s16b[:], s16b[:], musq[:], AO.subtract)
            sd = pp.tile([128, 16], F32)
            nc.vector.scalar_tensor_tensor(sd[:], s16b[:], BN_EPS, s16b[:],
                                           AO.add, AO.max)
            nc.scalar.activation(sd[:], sd[:], AF.Sqrt)
            rsd = pp.tile([128, 16], F32)
            nc.vector.reciprocal(rsd[:], sd[:])
            nc.vector.tensor_tensor(s16c[:], rsd[:], coef[:, 144:160], AO.mult)
            shf = pp.tile([128, 16], F32)
            nc.vector.tensor_tensor(shf[:], s16a[:], s16c[:], AO.mult)
            nc.vector.tensor_tensor(shf[:], coef[:, 160:176], shf[:], AO.subtract)
            nc.vector.tensor_tensor(
                hf[:], hf[:],
                s16c[:].rearrange("p (u f) -> p u f", u=1).to_broadcast(
                    [128, TPC, 16]), AO.mult)
            nc.vector.tensor_tensor(
                hf[:], hf[:],
                shf[:].rearrange("p (u f) -> p u f", u=1).to_broadcast(
                    [128, TPC, 16]), AO.add)
            nc.scalar.activation(h[:, :, 0:16], hf[:], AF.Relu)

            # ---------------- conv2 projections ----------------------------
            for j in range(5):
                bb = coef[:, 176 + j:177 + j].rearrange(
                    "p (t u) -> p t u", u=1).to_broadcast([128, TPC, 1])
                nc.vector.scalar_tensor_tensor(
                    vt[:, :, j:j + 1], h[:, :, 0:1],
                    cap(181 + j * 16 + 0), bb, AO.mult, AO.add)
                for i in range(1, 16):
                    nc.vector.scalar_tensor_tensor(
                        vt[:, :, j:j + 1], h[:, :, i:i + 1],
                        cap(181 + j * 16 + i), vt[:, :, j:j + 1],
                        AO.mult, AO.add)

            # ---------------- Horner chain ----------------------------------
            nc.scalar.activation(
                acc[:], vt[:, :, 4:5].rearrange("p t u -> p (t u)"), AF.Copy)
            nts = [ntAs, ntBs]
            for step, j in enumerate((3, 2, 1, 0)):
                nt = nts[step % 2]
                ntable_flush(acc[:], nt.ap())
                sweep_narrow(nt.ap())
                nc.vector.tensor_tensor(
                    acc[:], nar[:],
                    vt[:, :, j:j + 1].rearrange("p t u -> p (t u)"), AO.add)

            # ---------------- final linear + sigmoid ------------------------
            outb = pp.tile([128, TPC], F32)
            nc.scalar.activation(outb[:], acc[:], AF.Sigmoid,
                                 scale=cap(261), bias=cap(262))
            nc.sync.dma_start(out_d[:], outb[:])

    nc.compile()
    return nc


# ----------------------------------------------------------------------------
def kernel(x, edge_index, edge_weight, w1_init, w1_w, w1_root, w1_b,
           bn1_g, bn1_b, w2_init, w2_w, w2_root, w2_b, lin_w, lin_b):
    _install_hookshim()
    x = np.asarray(x, np.float32)
    edge_index = np.asarray(edge_index)
    edge_weight = np.asarray(edge_weight, np.float32)

    in_maps, layout, order = _host_prep(x, edge_index, edge_weight)

    # ---- coefficient packing (host): tiny-weight derived scalars
    w1_init = np.asarray(w1_init, np.float64)
    w1_w_ = np.asarray(w1_w, np.float64)
    w1_root = np.asarray(w1_root, np.float64)
    w1_b_ = np.asarray(w1_b, np.float64)
    w2_init = np.asarray(w2_init, np.float64)
    w2_w_ = np.asarray(w2_w, np.float64)
    w2_root = np.asarray(w2_root, np.float64)
    w2_b_ = np.asarray(w2_b, np.float64)

    coef = np.zeros(320, np.float64)
    coef[0:48] = w1_init[:, 0, :].reshape(-1)
    coef[48:96] = w1_root[:, 0, :].reshape(-1)
    coef[96:144] = w1_b_.reshape(-1)
    coef[144:160] = np.asarray(bn1_g, np.float64)
    coef[160:176] = np.asarray(bn1_b, np.float64)
    wk = w2_w_[:, 0, 0]
    gmat = np.zeros((5, 16), np.float64)
    beta = np.zeros(5, np.float64)
    gmat[4] = (wk ** 3 / 3.0) @ w2_init[:, :, 0]
    gmat[3] = (wk ** 3 / 3.0) @ w2_root[:, :, 0]; beta[3] = (wk ** 3 / 3.0) @ w2_b_[:, 0]
    gmat[2] = (wk ** 2 / 3.0) @ w2_root[:, :, 0]; beta[2] = (wk ** 2 / 3.0) @ w2_b_[:, 0]
    gmat[1] = (wk / 3.0) @ w2_root[:, :, 0];      beta[1] = (wk / 3.0) @ w2_b_[:, 0]
    gmat[0] = np.ones(3) / 3.0 @ w2_root[:, :, 0]; beta[0] = np.ones(3) / 3.0 @ w2_b_[:, 0]
    coef[176:181] = beta
    coef[181:261] = gmat.reshape(-1)
    coef[261] = np.asarray(lin_w, np.float64)[0, 0]
    coef[262] = np.asarray(lin_b, np.float64)[0]
    coef[263] = 1.0
    coef_np = np.tile(coef.astype(np.float32)[None, :], (128, 1))

    wrow = np.zeros(768, np.float64)
    for k in range(K):
        for o in range(16):
            wrow[(k * 16 + o) * 16:(k * 16 + o) * 16 + 16] = w1_w_[k, :, o]
    wrow_np = np.tile(wrow.astype(np.float32)[None, :], (128, 1))

    for m in in_maps:
        m["coef"] = coef_np
        m["wrow"] = wrow_np

    nc = _build_bass(layout)
    from concourse.bass_utils import run_bass_kernel_spmd
    trace = os.environ.get("BASS_GNN_TRACE", "0") == "1"
    res = run_bass_kernel_spmd(nc, in_maps, core_ids=list(range(NCORE)),
                               trace=trace)
    _EXEC_NS[0] = res.exec_time_ns

    out = np.empty((N, 1), np.float32)
    for c in range(NCORE):
        ob = res.results[c]["out"]        # [128, TPC]
        i = np.arange(ROWS)
        vals = ob[i % 128, i // 128]       # value at rank i
        keep = order[c] < SHARD
        out[c * SHARD + order[c][keep], 0] = vals[keep]
    return out


def last_exec_ns():
    return _EXEC_NS[0]
